# revision 40
# baseline (speedup 1.0000x reference)
"""MultiHeadAttention (pre-LN, residual) Trainium2 Bass kernel, 8 NeuronCores.

Problem: q,k,v [2, 2048, 1024], 16 heads x 64 dim, LN(q) -> QKV proj ->
softmax attention -> out proj -> +residual(q).

Sharding: core c owns tokens [512c, 512c+512) of the flattened [4096, 1024]
token axis (batch 0 = cores 0-3, batch 1 = cores 4-7).  Projections are
token-sharded; K^T / V are AllGathered (fp8, chunked) within each batch group
of 4 cores; each core runs attention + out projection for its 512 query
tokens over all 16 heads.

Matmuls are plain fp8 (1 output row / cycle; FWL weight loads).  S^T
contracts dk=64: the two heads of a pair run CONCURRENTLY in distinct PE
row-groups (tile_position (0,0) / (64,0)).  O^T contracts keys over 16
key-tiles with a 1/32-ones column (65th) in V producing the softmax
denominator in psum row 64.  exp() is split across engines by q columns:
ACT does true Exp into fp8e5m2 for q[0:QA]; DVE computes a Schraudolph-style
exp for q[QA:512] with one tensor_scalar (i8 = S*(4/ln2)/8 + 60) written
through an int8 bitcast of the e5m2 tile.  Normalization: approx-reciprocal
of psum row 64 + bf16 mask-matmul partition broadcast + DVE multiply.
"""

import numpy as np

N_CORES = 8
B, L, D = 2, 2048, 1024
H, DK, DV = 16, 64, 64
NT = B * L            # 4096 flattened tokens
TPC = NT // N_CORES   # 512 tokens per core
GROUP = 4             # cores per batch group
LB = L                # keys per batch (2048)
P = 128
NDT = D // P          # 8 d-tiles of 128
NTT = TPC // P        # 4 token tiles of 128 per core
NKT = LB // P         # 16 key tiles of 128 per batch
NKP = NKT // 2        # 8 key tile pairs
NHP = H // 2          # 8 head pairs
EPS = 1e-6

QA = 304              # q columns with true exp on ACT; rest Schraudolph on DVE
WS = 16.0             # host weight prescale into fp8 range
SCH_MUL = (4.0 / float(np.log(2.0))) / 8.0   # 0.72134752
SCH_ADD = 60.0        # e5m2 exponent bias 15 * 4

_CACHE = {}


def _np_reference(q, k, v, mask, w_q, w_k, w_v, w_o, ln_g, ln_b):
    """Pure-numpy fallback (only used if mask isn't all-ones)."""
    q64 = q.astype(np.float64)
    mu = q64.mean(-1, keepdims=True)
    var = q64.var(-1, keepdims=True)
    qn = (q64 - mu) / np.sqrt(var + EPS) * ln_g + ln_b
    Q = (qn @ w_q.T.astype(np.float64)).reshape(B, L, H, DK).transpose(0, 2, 1, 3)
    K = (k.astype(np.float64) @ w_k.T.astype(np.float64)).reshape(B, L, H, DK).transpose(0, 2, 1, 3)
    V = (v.astype(np.float64) @ w_v.T.astype(np.float64)).reshape(B, L, H, DV).transpose(0, 2, 1, 3)
    S = np.einsum("bhqd,bhkd->bhqk", Q / np.sqrt(DK), K)
    S = np.where(mask[None, None] == 0, -1e9, S)
    S = S - S.max(-1, keepdims=True)
    Pm = np.exp(S)
    Pm = Pm / Pm.sum(-1, keepdims=True)
    O = np.einsum("bhqk,bhkd->bhqd", Pm, V)
    O = O.transpose(0, 2, 1, 3).reshape(B, L, H * DV)
    out = O @ w_o.T.astype(np.float64) + q64
    return out.astype(np.float32)


def build_nc():
    import concourse.bass as bass
    import concourse.mybir as mybir
    import concourse.tile as tile
    from concourse import bacc
    from concourse.masks import make_identity

    f32 = mybir.dt.float32
    bf16 = mybir.dt.bfloat16
    f8e4 = mybir.dt.float8e4
    f8e5 = mybir.dt.float8e5
    i8 = mybir.dt.int8
    EXP = mybir.ActivationFunctionType.Exp

    nc = bacc.Bacc(num_devices=N_CORES)

    q_c = nc.declare_dram_parameter("q_c", [TPC, D], f32, isOutput=False)
    kT_c = nc.declare_dram_parameter("kT_c", [D, TPC], f8e4, isOutput=False)
    vT_c = nc.declare_dram_parameter("vT_c", [D, TPC], f8e4, isOutput=False)
    wq8 = nc.declare_dram_parameter("wq8", [D, D], f8e4, isOutput=False)
    wk8 = nc.declare_dram_parameter("wk8", [D, D], f8e4, isOutput=False)
    wv8 = nc.declare_dram_parameter("wv8", [D, D], f8e4, isOutput=False)
    wo8 = nc.declare_dram_parameter("wo8", [D, D], f8e4, isOutput=False)
    cq = nc.declare_dram_parameter("cq", [D], f32, isOutput=False)
    out_c = nc.declare_dram_parameter("out_c", [TPC, D], f32, isOutput=True)
    import os
    dbg = os.environ.get("KERNEL_DEBUG") == "1"
    if dbg:
        qnT_d = nc.declare_dram_parameter("qnT_d", [P, NDT, TPC], f8e4, isOutput=True)
        qT_d = nc.declare_dram_parameter("qT_d", [P, NHP, TPC], f8e4, isOutput=True)
        kT_d = nc.declare_dram_parameter("kT_d", [P, NHP, LB], f8e4, isOutput=True)
        v2_d = nc.declare_dram_parameter("v2_d", [P, H, NKT, 80], f8e5, isOutput=True)
        est_d = nc.declare_dram_parameter("est_d", [P, 4, NKT, TPC], f8e5, isOutput=True)
        aO_d = nc.declare_dram_parameter("aO_d", [P, NDT, TPC], f8e4, isOutput=True)
        den_d = nc.declare_dram_parameter("den_d", [1, H, TPC], f32, isOutput=True)
        r_d = nc.declare_dram_parameter("r_d", [65, NHP, TPC], f32, isOutput=True)

    RG = [[0, 1, 2, 3], [4, 5, 6, 7]]

    with tile.TileContext(nc) as tc:
        with tc.tile_pool(name="dram", bufs=1, space="DRAM") as dram:
            kag_in = [dram.tile([512, TPC], f8e4, name=f"kag_in{c}") for c in range(2)]
            kag_out = [dram.tile([GROUP, 512, TPC], f8e4, name=f"kag_out{c}") for c in range(2)]
            vag_in = dram.tile([TPC, D], f8e5, name="vag_in")
            vag_out = dram.tile([GROUP, TPC, D], f8e5, name="vag_out")
            warm_in = dram.tile([64], f8e4, name="warm_in")
            warm_out = dram.tile([GROUP, 64], f8e4, name="warm_out")

            with tc.tile_pool(name="singles", bufs=1) as singles:
                ident = singles.tile([P, P], bf16)
                make_identity(nc, ident)
                # partition-broadcast mask: row0 -> out parts 0:64, row64 -> 64:128
                bcm = singles.tile([65, P], bf16)
                nc.vector.memset(bcm[0:64, :], 0.0)
                nc.vector.memset(bcm[64:65, 0:DK], 0.0)
                nc.vector.memset(bcm[0:1, 0:DK], 1.0)
                nc.vector.memset(bcm[64:65, DK:P], 1.0)
                bco = singles.tile([1, P], bf16)
                nc.vector.memset(bco, 1.0)
                r2 = singles.tile([1, 2, TPC], f32)
                r2src = singles.tile([1, 2, TPC], f32)
                r2b = singles.tile([1, 2, TPC], bf16)
                cq_sb = singles.tile([P, NDT], f32)
                nc.sync.dma_start(out=cq_sb, in_=cq.rearrange("(t p) -> p t", p=P))
                eps_sb = singles.tile([P, 1], f32)
                nc.vector.memset(eps_sb, EPS)

                with tc.tile_pool(name="persist", bufs=1) as persist:
                    q_sb = persist.tile([P, NTT, D], f32)          # residual + LN in
                    qn_sb = persist.tile([P, NTT, D], bf16)        # LN out
                    qnT = persist.tile([P, NDT, TPC], f8e4)        # qn^T
                    qT = persist.tile([P, NHP, TPC], f8e4)         # Q^T by head pair
                    kT = persist.tile([P, NHP, LB], f8e4)          # K^T full batch
                    v2 = persist.tile([P, H, NKT, 80], f8e5)       # V + 1/32 ones col
                    aO = persist.tile([P, NDT, TPC], f8e4)         # attn out (x32/den)
                    kag_sb = persist.tile([P, NDT, TPC], f8e4)     # K^T pre-AG staging
                    vag_sb = persist.tile([P, NTT, D], f8e5)       # V pre-AG staging
                    wk_sb = persist.tile([P, NDT, D], f8e4)
                    wv_sb = persist.tile([P, NDT, D], f8e4)
                    wq_sb = persist.tile([P, NDT, D], f8e4)
                    wo_sb = persist.tile([P, NDT, D], f8e4)
                    kc_sb = persist.tile([P, NDT, TPC], f8e4)      # k^T shard
                    vc_sb = persist.tile([P, NDT, TPC], f8e4)      # v^T shard

                    nc.gpsimd.collective_compute(
                        "AllGather", mybir.AluOpType.bypass,
                        replica_groups=RG,
                        ins=[warm_in[:].opt()],
                        outs=[warm_out[:, :].opt()],
                    )
                    # ones column of V (denominator): 1/32 so r = 32/den
                    nc.gpsimd.memset(v2[:, :, :, 64:65], 1.0 / 32.0)
                    nc.gpsimd.memset(v2[:, :, :, 65:66], 0.0)

                    # ---- input DMAs (K-proj operands first) ----
                    nc.sync.dma_start(out=wk_sb, in_=wk8.rearrange("(dt p) f -> p dt f", p=P))
                    nc.sync.dma_start(out=kc_sb, in_=kT_c.rearrange("(dt p) t -> p dt t", p=P))
                    nc.sync.dma_start(out=q_sb, in_=q_c.rearrange("(tt p) d -> p tt d", p=P))

                    # =========== Phase 1: LN + K proj/AG + Q proj + V proj/AG ===
                    with tc.tile_pool(name="p1s", bufs=4) as p1s, \
                         tc.tile_pool(name="ppsum", bufs=3, space="PSUM") as ppsum, \
                         tc.tile_pool(name="tpsum", bufs=2, space="PSUM") as tpsum:

                        # K projection: psum tile t = features [128t,128t+128)
                        for t in range(NDT):
                            ps = ppsum.tile([P, TPC], f32, tag="pp")
                            for dt in range(NDT):
                                nc.tensor.matmul(
                                    ps,
                                    wk_sb[:, dt, t * P:(t + 1) * P],
                                    kc_sb[:, dt, :],
                                    start=(dt == 0), stop=(dt == NDT - 1),
                                )
                            nc.scalar.mul(kag_sb[:, t, :], ps, 1.0 / WS)
                            if t % 4 == 3:
                                c = t // 4
                                nc.gpsimd.dma_start(
                                    out=kag_in[c].rearrange("(hp p) t -> p hp t", p=P),
                                    in_=kag_sb[:, 4 * c:4 * c + 4, :],
                                )
                                nc.gpsimd.collective_compute(
                                    "AllGather", mybir.AluOpType.bypass,
                                    replica_groups=RG,
                                    ins=[kag_in[c][:, :].opt()],
                                    outs=[kag_out[c][:, :, :].opt()],
                                )
                                for r in range(GROUP):
                                    nc.sync.dma_start(
                                        out=kT[:, 4 * c:4 * c + 4, r * TPC:(r + 1) * TPC],
                                        in_=kag_out[c][r].rearrange("(hp p) t -> p hp t", p=P),
                                    )

                        # LayerNorm on DVE (overlaps K proj on PE)
                        for tt in range(NTT):
                            stats = p1s.tile([P, 2, 6], f32, tag="st")
                            for sg in range(2):
                                nc.vector.bn_stats(
                                    out=stats[:, sg, :],
                                    in_=q_sb[:, tt, sg * 512:(sg + 1) * 512],
                                )
                            mv = p1s.tile([P, 2], f32, tag="mv")
                            nc.vector.bn_aggr(out=mv, in_=stats)
                            rstd = p1s.tile([P, 1], f32, tag="rs")
                            nc.scalar.activation(
                                out=rstd, in_=mv[:, 1:2],
                                func=mybir.ActivationFunctionType.Sqrt,
                                bias=eps_sb, scale=1.0,
                            )
                            nc.vector.reciprocal(out=rstd, in_=rstd)
                            nc.vector.tensor_scalar(
                                out=qn_sb[:, tt, :], in0=q_sb[:, tt, :],
                                scalar1=mv[:, 0:1], scalar2=rstd,
                                op0=mybir.AluOpType.subtract,
                                op1=mybir.AluOpType.mult,
                            )

                        nc.sync.dma_start(out=wv_sb, in_=wv8.rearrange("(dt p) f -> p dt f", p=P))
                        nc.sync.dma_start(out=vc_sb, in_=vT_c.rearrange("(dt p) t -> p dt t", p=P))
                        # V projection: psum tile (fc, tt) = [128 tok, 512 feat]
                        for fc in range(2):
                            for tt in range(NTT):
                                ps = ppsum.tile([P, TPC], f32, tag="pp")
                                for dt in range(NDT):
                                    nc.tensor.matmul(
                                        ps,
                                        vc_sb[:, dt, tt * P:(tt + 1) * P],
                                        wv_sb[:, dt, fc * 512:(fc + 1) * 512],
                                        start=(dt == 0), stop=(dt == NDT - 1),
                                    )
                                nc.vector.tensor_scalar_mul(
                                    vag_sb[:, tt, fc * 512:(fc + 1) * 512], ps, 1.0 / WS
                                )
                        nc.gpsimd.dma_start(
                            out=vag_in.rearrange("(tt p) f -> p tt f", p=P),
                            in_=vag_sb,
                        )
                        nc.gpsimd.collective_compute(
                            "AllGather", mybir.AluOpType.bypass,
                            replica_groups=RG,
                            ins=[vag_in[:, :].opt()],
                            outs=[vag_out[:, :, :].opt()],
                        )
                        # v2[p, h, kt, dv]; key = g*512 + tt*128 + p
                        nc.sync.dma_start(
                            out=v2[:, :, :, 0:DV],
                            in_=vag_out.rearrange(
                                "g (tt p) (h dv) -> p h (g tt) dv",
                                p=P, dv=DV,
                            ),
                        )

                        nc.sync.dma_start(out=wq_sb, in_=wq8.rearrange("(dt p) f -> p dt f", p=P))
                        nc.sync.dma_start(out=wo_sb, in_=wo8.rearrange("(dt p) m -> p dt m", p=P))
                        # qn^T: PE transpose (bf16) + ACT evict to fp8
                        for dt in range(NDT):
                            tp = tpsum.tile([P, TPC], bf16, tag="tp")
                            for tt in range(NTT):
                                nc.tensor.transpose(
                                    tp[:, tt * P:(tt + 1) * P],
                                    qn_sb[:, tt, dt * P:(dt + 1) * P],
                                    ident,
                                )
                            nc.scalar.mul(qnT[:, dt, :], tp, 1.0)

                        # Q projection + bias
                        for t in range(NDT):
                            ps = ppsum.tile([P, TPC], f32, tag="pp")
                            for dt in range(NDT):
                                nc.tensor.matmul(
                                    ps,
                                    wq_sb[:, dt, t * P:(t + 1) * P],
                                    qnT[:, dt, :],
                                    start=(dt == 0), stop=(dt == NDT - 1),
                                )
                            nc.scalar.activation(
                                out=qT[:, t, :], in_=ps,
                                func=mybir.ActivationFunctionType.Identity,
                                bias=cq_sb[:, t:t + 1], scale=1.0 / WS,
                            )

                    # =========== Phase 2: attention ============================
                    if dbg:
                        den_dbg = persist.tile([1, H, TPC], f32)
                        r_dbg = persist.tile([65, NHP, TPC], f32)
                    with tc.tile_pool(name="est", bufs=1) as estp, \
                         tc.tile_pool(name="rbp", bufs=2) as rbp, \
                         tc.tile_pool(name="spsum", bufs=1, space="PSUM") as spsum, \
                         tc.tile_pool(name="opsum", bufs=3, space="PSUM") as opsum, \
                         tc.tile_pool(name="rpsum", bufs=1, space="PSUM") as rpsum:

                        est_bufs = [estp.tile([P, NKT, TPC], f8e5, name=f"est{j}")
                                    for j in range(4)]
                        o_ps = {}

                        def emit_o(h):
                            est = est_bufs[h % 4]
                            ops = opsum.tile([P, TPC], f32, tag="o", name=f"o_{h}")
                            for kt in range(NKT):
                                nc.tensor.matmul(
                                    ops[0:DV + 2, :],
                                    v2[:, h, kt, 0:DV + 2],
                                    est[:, kt, :],
                                    start=(kt == 0), stop=(kt == NKT - 1),
                                )
                            o_ps[h] = ops

                        def emit_norm(h):
                            # h odd: normalize heads h-1, h
                            opsA, opsB = o_ps.pop(h - 1), o_ps.pop(h)
                            if dbg:
                                nc.vector.tensor_copy(den_dbg[0:1, h - 1, :], opsA[DV:DV + 1, :])
                                nc.vector.tensor_copy(den_dbg[0:1, h, :], opsB[DV:DV + 1, :])
                            nc.vector.tensor_copy(r2src[0:1, 0, :], opsA[DV:DV + 1, :])
                            nc.vector.tensor_copy(r2src[0:1, 1, :], opsB[DV:DV + 1, :])
                            nc.vector.reciprocal_approx_fast(
                                out=r2[0:1, 0, :], in_=r2src[0:1, 0, :])
                            nc.vector.reciprocal_approx_fast(
                                out=r2[0:1, 1, :], in_=r2src[0:1, 1, :])
                            if dbg:
                                nc.vector.tensor_copy(r_dbg[0:1, h // 2, :], r2[0:1, 0, :])
                                nc.vector.tensor_copy(r_dbg[64:65, h // 2, :], r2[0:1, 1, :])
                            nc.vector.tensor_copy(r2b[:, :, :], r2[:, :, :])
                            d = h // 2
                            rbcA = rpsum.tile([P, TPC], f32, tag="rb", name=f"rbA_{h}")
                            nc.tensor.matmul(
                                rbcA[:, :], bco[:, :], r2b[0:1, 0, :],
                                start=True, stop=True,
                            )
                            rbsA = rbp.tile([P, TPC], bf16, tag="rs", name=f"rsA_{h}")
                            nc.scalar.mul(rbsA[:, :], rbcA[:, :], 1.0)
                            nc.vector.tensor_tensor(
                                out=aO[0:DV, d, :], in0=opsA[0:DV, :],
                                in1=rbsA[0:DV, :], op=mybir.AluOpType.mult,
                            )
                            rbcB = rpsum.tile([P, TPC], f32, tag="rb", name=f"rbB_{h}")
                            nc.tensor.matmul(
                                rbcB[:, :], bco[:, :], r2b[0:1, 1, :],
                                start=True, stop=True,
                            )
                            rbsB = rbp.tile([P, TPC], bf16, tag="rs", name=f"rsB_{h}")
                            nc.scalar.mul(rbsB[:, :], rbcB[:, :], 1.0)
                            nc.vector.tensor_tensor(
                                out=aO[DV:P, d, :], in0=opsB[0:DV, :],
                                in1=rbsB[DV:P, :], op=mybir.AluOpType.mult,
                            )

                        def emit_hp(hp):
                            # S/exp for pair hp, with O matmuls of pair hp-1
                            # interleaved ahead of each S pair so PE never
                            # stalls on the exp ping-pong
                            ests = [est_bufs[(2 * hp) % 4], est_bufs[(2 * hp + 1) % 4]]
                            if hp > 0:
                                opsP = [
                                    opsum.tile([P, TPC], f32, tag="o",
                                               name=f"o_{2 * hp - 2}"),
                                    opsum.tile([P, TPC], f32, tag="o",
                                               name=f"o_{2 * hp - 1}"),
                                ]
                                estP = [est_bufs[(2 * hp - 2) % 4],
                                        est_bufs[(2 * hp - 1) % 4]]
                            for tp2 in range(NKP):
                                if hp > 0:
                                    for j in range(2):
                                        hP = 2 * hp - 2 + j
                                        for half in range(2):
                                            kt = 2 * tp2 + half
                                            nc.tensor.matmul(
                                                opsP[j][0:DV + 2, :],
                                                v2[:, hP, kt, 0:DV + 2],
                                                estP[j][:, kt, :],
                                                start=(kt == 0), stop=(kt == NKT - 1),
                                            )
                                sps = [
                                    spsum.tile([P, 2, TPC], f32, tag="sA",
                                               name=f"sA_{hp}_{tp2}"),
                                    spsum.tile([P, 2, TPC], f32, tag="sB",
                                               name=f"sB_{hp}_{tp2}"),
                                ]
                                for half in range(2):
                                    kt = 2 * tp2 + half
                                    for par in range(2):
                                        nc.tensor.matmul(
                                            sps[par][:, half, :],
                                            kT[64 * par:64 * par + 64, hp,
                                               kt * P:(kt + 1) * P],
                                            qT[64 * par:64 * par + 64, hp, :],
                                            start=True, stop=True,
                                            tile_position=(64 * par, 0),
                                        )
                                for par in range(2):
                                    if QA > 0:
                                        nc.scalar.activation(
                                            out=ests[par][:, 2 * tp2:2 * tp2 + 2, 0:QA],
                                            in_=sps[par][:, :, 0:QA],
                                            func=EXP, scale=0.125,
                                        )
                                    if QA < TPC:
                                        nc.vector.tensor_scalar(
                                            out=ests[par][:, 2 * tp2:2 * tp2 + 2, QA:TPC].bitcast(i8),
                                            in0=sps[par][:, :, QA:TPC],
                                            scalar1=SCH_MUL, scalar2=SCH_ADD,
                                            op0=mybir.AluOpType.mult,
                                            op1=mybir.AluOpType.add,
                                        )
                            if hp > 0:
                                o_ps[2 * hp - 2] = opsP[0]
                                o_ps[2 * hp - 1] = opsP[1]
                                emit_norm(2 * hp - 1)

                        for hp in range(NHP):
                            emit_hp(hp)
                        emit_o(H - 2)
                        emit_o(H - 1)
                        emit_norm(H - 1)

                        if dbg:
                            nc.sync.dma_start(out=qnT_d[:, :, :], in_=qnT)
                            nc.sync.dma_start(out=qT_d[:, :, :], in_=qT)
                            nc.sync.dma_start(out=kT_d[:, :, :], in_=kT)
                            nc.sync.dma_start(out=v2_d[:, :, :, :], in_=v2)
                            for j in range(4):
                                nc.sync.dma_start(out=est_d[:, j, :, :], in_=est_bufs[j])
                            nc.sync.dma_start(out=aO_d[:, :, :], in_=aO)
                            nc.sync.dma_start(out=den_d[:, :, :], in_=den_dbg)
                            nc.sync.dma_start(out=r_d[:, :, :], in_=r_dbg)

                    # =========== Phase 3: out projection + residual ============
                    with tc.tile_pool(name="p4o", bufs=2) as p4o, \
                         tc.tile_pool(name="fpsum", bufs=2, space="PSUM") as fpsum:
                        for tt in range(NTT):
                            for mc in range(2):
                                fps = fpsum.tile([P, TPC], f32, tag="f")
                                for dt in range(NDT):
                                    nc.tensor.matmul(
                                        fps,
                                        aO[:, dt, tt * P:(tt + 1) * P],
                                        wo_sb[:, dt, mc * 512:(mc + 1) * 512],
                                        start=(dt == 0), stop=(dt == NDT - 1),
                                    )
                                ob = p4o.tile([P, TPC], f32, tag="ob")
                                nc.vector.scalar_tensor_tensor(
                                    out=ob, in0=fps, scalar=1.0 / (WS * 32.0),
                                    in1=q_sb[:, tt, mc * 512:(mc + 1) * 512],
                                    op0=mybir.AluOpType.mult,
                                    op1=mybir.AluOpType.add,
                                )
                                nc.sync.dma_start(
                                    out=out_c[tt * P:(tt + 1) * P, mc * 512:(mc + 1) * 512],
                                    in_=ob,
                                )

    nc.compile()
    return nc


def _get_nc():
    if "nc" not in _CACHE:
        _CACHE["nc"] = build_nc()
    return _CACHE["nc"]


def make_in_maps(q, k, v, w_q, w_k, w_v, w_o, ln_g, ln_b):
    import ml_dtypes

    e4 = ml_dtypes.float8_e4m3
    q2 = np.ascontiguousarray(q.reshape(NT, D), dtype=np.float32)
    kT8 = np.ascontiguousarray(k.reshape(NT, D).T.astype(e4))
    vT8 = np.ascontiguousarray(v.reshape(NT, D).T.astype(e4))
    wgq = w_q * ln_g[None, :]
    wq8 = np.ascontiguousarray((WS * wgq).T.astype(e4))
    wk8 = np.ascontiguousarray((WS * w_k).T.astype(e4))
    wv8 = np.ascontiguousarray((WS * w_v).T.astype(e4))
    wo8 = np.ascontiguousarray((WS * w_o).T.astype(e4))
    cq = np.ascontiguousarray(w_q @ ln_b, dtype=np.float32)
    in_maps = []
    for c in range(N_CORES):
        sl = slice(c * TPC, (c + 1) * TPC)
        in_maps.append(
            {
                "q_c": q2[sl],
                "kT_c": np.ascontiguousarray(kT8[:, sl]),
                "vT_c": np.ascontiguousarray(vT8[:, sl]),
                "wq8": wq8,
                "wk8": wk8,
                "wv8": wv8,
                "wo8": wo8,
                "cq": cq,
            }
        )
    return in_maps


def run(inputs, trace=False, tmpdir=None):
    """Run the device kernel.  Returns (out [B, L, D], BassKernelResults)."""
    from concourse.bass_utils import run_bass_kernel_spmd

    nc = _get_nc()
    in_maps = make_in_maps(
        inputs["q"], inputs["k"], inputs["v"], inputs["w_q"], inputs["w_k"],
        inputs["w_v"], inputs["w_o"], inputs["ln_g"], inputs["ln_b"],
    )
    res = run_bass_kernel_spmd(
        nc, in_maps, list(range(N_CORES)), trace=trace, tmpdir=tmpdir
    )
    rows = np.concatenate([res.results[c]["out_c"] for c in range(N_CORES)], axis=0)
    return rows.reshape(B, L, D), res


def kernel(q, k, v, mask, w_q, w_k, w_v, w_o, ln_g, ln_b):
    q = np.asarray(q, dtype=np.float32)
    k = np.asarray(k, dtype=np.float32)
    v = np.asarray(v, dtype=np.float32)
    mask = np.asarray(mask)
    w_q = np.asarray(w_q, dtype=np.float32)
    w_k = np.asarray(w_k, dtype=np.float32)
    w_v = np.asarray(w_v, dtype=np.float32)
    w_o = np.asarray(w_o, dtype=np.float32)
    ln_g = np.asarray(ln_g, dtype=np.float32)
    ln_b = np.asarray(ln_b, dtype=np.float32)
    if not np.all(mask == 1):
        return _np_reference(q, k, v, mask, w_q, w_k, w_v, w_o, ln_g, ln_b)
    out, _ = run(
        {"q": q, "k": k, "v": v, "w_q": w_q, "w_k": w_k, "w_v": w_v,
         "w_o": w_o, "ln_g": ln_g, "ln_b": ln_b},
        trace=False,
    )
    return out


# revision 41
# speedup vs baseline: 1.0645x; 1.0645x over previous
"""MultiHeadAttention (pre-LN, residual) Trainium2 Bass kernel, 8 NeuronCores.

Problem: q,k,v [2, 2048, 1024], 16 heads x 64 dim, LN(q) -> QKV proj ->
softmax attention -> out proj -> +residual(q).

Sharding: core c owns tokens [512c, 512c+512) of the flattened [4096, 1024]
token axis (batch 0 = cores 0-3, batch 1 = cores 4-7).  Projections are
token-sharded; K^T / V are AllGathered (fp8, chunked) within each batch group
of 4 cores; each core runs attention + out projection for its 512 query
tokens over all 16 heads.

Matmuls are plain fp8 (1 output row / cycle; FWL weight loads).  S^T
contracts dk=64: the two heads of a pair run CONCURRENTLY in distinct PE
row-groups (tile_position (0,0) / (64,0)).  O^T contracts keys over 16
key-tiles with a 1/32-ones column (65th) in V producing the softmax
denominator in psum row 64.  exp() is split across engines by q columns:
ACT does true Exp into fp8e5m2 for q[0:QA]; DVE computes a Schraudolph-style
exp for q[QA:512] with one tensor_scalar (i8 = S*(4/ln2)/8 + 60) written
through an int8 bitcast of the e5m2 tile.  Normalization: approx-reciprocal
of psum row 64 + bf16 mask-matmul partition broadcast + DVE multiply.
"""

import numpy as np

N_CORES = 8
B, L, D = 2, 2048, 1024
H, DK, DV = 16, 64, 64
NT = B * L            # 4096 flattened tokens
TPC = NT // N_CORES   # 512 tokens per core
GROUP = 4             # cores per batch group
LB = L                # keys per batch (2048)
P = 128
NDT = D // P          # 8 d-tiles of 128
NTT = TPC // P        # 4 token tiles of 128 per core
NKT = LB // P         # 16 key tiles of 128 per batch
NKP = NKT // 2        # 8 key tile pairs
NHP = H // 2          # 8 head pairs
EPS = 1e-6

QA = 304              # q columns with true exp on ACT; rest Schraudolph on DVE
WS = 16.0             # host weight prescale into fp8 range
SCH_MUL = (4.0 / float(np.log(2.0))) / 8.0   # 0.72134752
SCH_ADD = 60.0        # e5m2 exponent bias 15 * 4

_CACHE = {}


def _np_reference(q, k, v, mask, w_q, w_k, w_v, w_o, ln_g, ln_b):
    """Pure-numpy fallback (only used if mask isn't all-ones)."""
    q64 = q.astype(np.float64)
    mu = q64.mean(-1, keepdims=True)
    var = q64.var(-1, keepdims=True)
    qn = (q64 - mu) / np.sqrt(var + EPS) * ln_g + ln_b
    Q = (qn @ w_q.T.astype(np.float64)).reshape(B, L, H, DK).transpose(0, 2, 1, 3)
    K = (k.astype(np.float64) @ w_k.T.astype(np.float64)).reshape(B, L, H, DK).transpose(0, 2, 1, 3)
    V = (v.astype(np.float64) @ w_v.T.astype(np.float64)).reshape(B, L, H, DV).transpose(0, 2, 1, 3)
    S = np.einsum("bhqd,bhkd->bhqk", Q / np.sqrt(DK), K)
    S = np.where(mask[None, None] == 0, -1e9, S)
    S = S - S.max(-1, keepdims=True)
    Pm = np.exp(S)
    Pm = Pm / Pm.sum(-1, keepdims=True)
    O = np.einsum("bhqk,bhkd->bhqd", Pm, V)
    O = O.transpose(0, 2, 1, 3).reshape(B, L, H * DV)
    out = O @ w_o.T.astype(np.float64) + q64
    return out.astype(np.float32)


def build_nc():
    import concourse.bass as bass
    import concourse.mybir as mybir
    import concourse.tile as tile
    from concourse import bacc
    from concourse.masks import make_identity

    f32 = mybir.dt.float32
    bf16 = mybir.dt.bfloat16
    f8e4 = mybir.dt.float8e4
    f8e5 = mybir.dt.float8e5
    i8 = mybir.dt.int8
    EXP = mybir.ActivationFunctionType.Exp

    nc = bacc.Bacc(num_devices=N_CORES)

    q_c = nc.declare_dram_parameter("q_c", [TPC, D], f32, isOutput=False)
    kT_c = nc.declare_dram_parameter("kT_c", [D, TPC], f8e4, isOutput=False)
    vT_c = nc.declare_dram_parameter("vT_c", [D, TPC], f8e4, isOutput=False)
    wq8 = nc.declare_dram_parameter("wq8", [D, D], f8e4, isOutput=False)
    wk8 = nc.declare_dram_parameter("wk8", [D, D], f8e4, isOutput=False)
    wv8 = nc.declare_dram_parameter("wv8", [D, D], f8e4, isOutput=False)
    wo8 = nc.declare_dram_parameter("wo8", [D, D], f8e4, isOutput=False)
    cq = nc.declare_dram_parameter("cq", [D], f32, isOutput=False)
    out_c = nc.declare_dram_parameter("out_c", [TPC, D], f32, isOutput=True)
    import os
    dbg = os.environ.get("KERNEL_DEBUG") == "1"
    if dbg:
        qnT_d = nc.declare_dram_parameter("qnT_d", [P, NDT, TPC], f8e4, isOutput=True)
        qT_d = nc.declare_dram_parameter("qT_d", [P, NHP, TPC], f8e4, isOutput=True)
        kT_d = nc.declare_dram_parameter("kT_d", [P, NHP, LB], f8e4, isOutput=True)
        v2_d = nc.declare_dram_parameter("v2_d", [P, H, NKT, 80], f8e5, isOutput=True)
        est_d = nc.declare_dram_parameter("est_d", [P, 4, NKT, TPC], f8e5, isOutput=True)
        aO_d = nc.declare_dram_parameter("aO_d", [P, NDT, TPC], f8e4, isOutput=True)
        den_d = nc.declare_dram_parameter("den_d", [1, H, TPC], f32, isOutput=True)
        r_d = nc.declare_dram_parameter("r_d", [65, NHP, TPC], f32, isOutput=True)

    RG = [[0, 1, 2, 3], [4, 5, 6, 7]]

    with tile.TileContext(nc) as tc:
        with tc.tile_pool(name="dram", bufs=1, space="DRAM") as dram:
            kag_in = dram.tile([D, TPC], f8e4, name="kag_in")
            kag_out = dram.tile([GROUP, D, TPC], f8e4, name="kag_out")
            vag_in = dram.tile([TPC, D], f8e5, name="vag_in")
            vag_out = dram.tile([GROUP, TPC, D], f8e5, name="vag_out")
            warm_in = dram.tile([64], f8e4, name="warm_in")
            warm_out = dram.tile([GROUP, 64], f8e4, name="warm_out")

            with tc.tile_pool(name="singles", bufs=1) as singles:
                ident = singles.tile([P, P], bf16)
                make_identity(nc, ident)
                # partition-broadcast mask: row0 -> out parts 0:64, row64 -> 64:128
                bcm = singles.tile([65, P], bf16)
                nc.vector.memset(bcm[0:64, :], 0.0)
                nc.vector.memset(bcm[64:65, 0:DK], 0.0)
                nc.vector.memset(bcm[0:1, 0:DK], 1.0)
                nc.vector.memset(bcm[64:65, DK:P], 1.0)
                bco = singles.tile([1, P], bf16)
                nc.vector.memset(bco, 1.0)
                r2 = singles.tile([1, 2, TPC], f32)
                r2src = singles.tile([1, 2, TPC], f32)
                r2b = singles.tile([1, 2, TPC], bf16)
                cq_sb = singles.tile([P, NDT], f32)
                nc.sync.dma_start(out=cq_sb, in_=cq.rearrange("(t p) -> p t", p=P))
                eps_sb = singles.tile([P, 1], f32)
                nc.vector.memset(eps_sb, EPS)

                with tc.tile_pool(name="persist", bufs=1) as persist:
                    q_sb = persist.tile([P, NTT, D], f32)          # residual + LN in
                    qn_sb = persist.tile([P, NTT, D], bf16)        # LN out
                    qnT = persist.tile([P, NDT, TPC], f8e4)        # qn^T
                    qT = persist.tile([P, NHP, TPC], f8e4)         # Q^T by head pair
                    kT = persist.tile([P, NHP, LB], f8e4)          # K^T full batch
                    v2 = persist.tile([P, H, NKT, 80], f8e5)       # V + 1/32 ones col
                    aO = persist.tile([P, NDT, TPC], f8e4)         # attn out (x32/den)
                    kag_sb = persist.tile([P, NDT, TPC], f8e4)     # K^T pre-AG staging
                    vag_sb = persist.tile([P, NTT, D], f8e5)       # V pre-AG staging
                    wk_sb = persist.tile([P, NDT, D], f8e4)
                    wv_sb = persist.tile([P, NDT, D], f8e4)
                    wq_sb = persist.tile([P, NDT, D], f8e4)
                    wo_sb = persist.tile([P, NDT, D], f8e4)
                    kc_sb = persist.tile([P, NDT, TPC], f8e4)      # k^T shard
                    vc_sb = persist.tile([P, NDT, TPC], f8e4)      # v^T shard

                    nc.gpsimd.collective_compute(
                        "AllGather", mybir.AluOpType.bypass,
                        replica_groups=RG,
                        ins=[warm_in[:].opt()],
                        outs=[warm_out[:, :].opt()],
                    )
                    # ones column of V (denominator): 1/32 so r = 32/den
                    nc.gpsimd.memset(v2[:, :, :, 64:65], 1.0 / 32.0)
                    nc.gpsimd.memset(v2[:, :, :, 65:66], 0.0)

                    # ---- input DMAs (K-proj operands first) ----
                    nc.sync.dma_start(out=wk_sb, in_=wk8.rearrange("(dt p) f -> p dt f", p=P))
                    nc.sync.dma_start(out=kc_sb, in_=kT_c.rearrange("(dt p) t -> p dt t", p=P))
                    nc.sync.dma_start(out=q_sb, in_=q_c.rearrange("(tt p) d -> p tt d", p=P))

                    # =========== Phase 1: LN + K proj/AG + Q proj + V proj/AG ===
                    with tc.tile_pool(name="p1s", bufs=4) as p1s, \
                         tc.tile_pool(name="ppsum", bufs=3, space="PSUM") as ppsum, \
                         tc.tile_pool(name="tpsum", bufs=2, space="PSUM") as tpsum:

                        # K projection: psum tile t = features [128t,128t+128)
                        for t in range(NDT):
                            ps = ppsum.tile([P, TPC], f32, tag="pp")
                            for dt in range(NDT):
                                nc.tensor.matmul(
                                    ps,
                                    wk_sb[:, dt, t * P:(t + 1) * P],
                                    kc_sb[:, dt, :],
                                    start=(dt == 0), stop=(dt == NDT - 1),
                                )
                            nc.scalar.mul(kag_sb[:, t, :], ps, 1.0 / WS)
                        nc.gpsimd.dma_start(
                            out=kag_in.rearrange("(hp p) t -> p hp t", p=P),
                            in_=kag_sb,
                        )
                        nc.gpsimd.collective_compute(
                            "AllGather", mybir.AluOpType.bypass,
                            replica_groups=RG,
                            ins=[kag_in[:, :].opt()],
                            outs=[kag_out[:, :, :].opt()],
                        )
                        for r in range(GROUP):
                            nc.sync.dma_start(
                                out=kT[:, :, r * TPC:(r + 1) * TPC],
                                in_=kag_out[r].rearrange("(hp p) t -> p hp t", p=P),
                            )

                        # LayerNorm on DVE (overlaps K proj on PE)
                        for tt in range(NTT):
                            stats = p1s.tile([P, 2, 6], f32, tag="st")
                            for sg in range(2):
                                nc.vector.bn_stats(
                                    out=stats[:, sg, :],
                                    in_=q_sb[:, tt, sg * 512:(sg + 1) * 512],
                                )
                            mv = p1s.tile([P, 2], f32, tag="mv")
                            nc.vector.bn_aggr(out=mv, in_=stats)
                            rstd = p1s.tile([P, 1], f32, tag="rs")
                            nc.scalar.activation(
                                out=rstd, in_=mv[:, 1:2],
                                func=mybir.ActivationFunctionType.Sqrt,
                                bias=eps_sb, scale=1.0,
                            )
                            nc.vector.reciprocal(out=rstd, in_=rstd)
                            nc.vector.tensor_scalar(
                                out=qn_sb[:, tt, :], in0=q_sb[:, tt, :],
                                scalar1=mv[:, 0:1], scalar2=rstd,
                                op0=mybir.AluOpType.subtract,
                                op1=mybir.AluOpType.mult,
                            )

                        nc.sync.dma_start(out=wv_sb, in_=wv8.rearrange("(dt p) f -> p dt f", p=P))
                        nc.sync.dma_start(out=vc_sb, in_=vT_c.rearrange("(dt p) t -> p dt t", p=P))
                        # V projection: psum tile (fc, tt) = [128 tok, 512 feat]
                        for fc in range(2):
                            for tt in range(NTT):
                                ps = ppsum.tile([P, TPC], f32, tag="pp")
                                for dt in range(NDT):
                                    nc.tensor.matmul(
                                        ps,
                                        vc_sb[:, dt, tt * P:(tt + 1) * P],
                                        wv_sb[:, dt, fc * 512:(fc + 1) * 512],
                                        start=(dt == 0), stop=(dt == NDT - 1),
                                    )
                                nc.vector.tensor_scalar_mul(
                                    vag_sb[:, tt, fc * 512:(fc + 1) * 512], ps, 1.0 / WS
                                )
                        nc.gpsimd.dma_start(
                            out=vag_in.rearrange("(tt p) f -> p tt f", p=P),
                            in_=vag_sb,
                        )
                        nc.gpsimd.collective_compute(
                            "AllGather", mybir.AluOpType.bypass,
                            replica_groups=RG,
                            ins=[vag_in[:, :].opt()],
                            outs=[vag_out[:, :, :].opt()],
                        )
                        # v2[p, h, kt, dv]; key = g*512 + tt*128 + p
                        nc.sync.dma_start(
                            out=v2[:, :, :, 0:DV],
                            in_=vag_out.rearrange(
                                "g (tt p) (h dv) -> p h (g tt) dv",
                                p=P, dv=DV,
                            ),
                        )

                        nc.sync.dma_start(out=wq_sb, in_=wq8.rearrange("(dt p) f -> p dt f", p=P))
                        nc.sync.dma_start(out=wo_sb, in_=wo8.rearrange("(dt p) m -> p dt m", p=P))
                        # qn^T: PE transpose (bf16) + ACT evict to fp8
                        for dt in range(NDT):
                            tp = tpsum.tile([P, TPC], bf16, tag="tp")
                            for tt in range(NTT):
                                nc.tensor.transpose(
                                    tp[:, tt * P:(tt + 1) * P],
                                    qn_sb[:, tt, dt * P:(dt + 1) * P],
                                    ident,
                                )
                            nc.scalar.mul(qnT[:, dt, :], tp, 1.0)

                        # Q projection + bias
                        for t in range(NDT):
                            ps = ppsum.tile([P, TPC], f32, tag="pp")
                            for dt in range(NDT):
                                nc.tensor.matmul(
                                    ps,
                                    wq_sb[:, dt, t * P:(t + 1) * P],
                                    qnT[:, dt, :],
                                    start=(dt == 0), stop=(dt == NDT - 1),
                                )
                            nc.scalar.activation(
                                out=qT[:, t, :], in_=ps,
                                func=mybir.ActivationFunctionType.Identity,
                                bias=cq_sb[:, t:t + 1], scale=1.0 / WS,
                            )

                    # =========== Phase 2: attention ============================
                    if dbg:
                        den_dbg = persist.tile([1, H, TPC], f32)
                        r_dbg = persist.tile([65, NHP, TPC], f32)
                    with tc.tile_pool(name="est", bufs=1) as estp, \
                         tc.tile_pool(name="rbp", bufs=2) as rbp, \
                         tc.tile_pool(name="spsum", bufs=1, space="PSUM") as spsum, \
                         tc.tile_pool(name="opsum", bufs=3, space="PSUM") as opsum, \
                         tc.tile_pool(name="rpsum", bufs=1, space="PSUM") as rpsum:

                        est_bufs = [estp.tile([P, NKT, TPC], f8e5, name=f"est{j}")
                                    for j in range(4)]
                        o_ps = {}

                        def emit_o(h):
                            est = est_bufs[h % 4]
                            ops = opsum.tile([P, TPC], f32, tag="o", name=f"o_{h}")
                            for kt in range(NKT):
                                nc.tensor.matmul(
                                    ops[0:DV + 2, :],
                                    v2[:, h, kt, 0:DV + 2],
                                    est[:, kt, :],
                                    start=(kt == 0), stop=(kt == NKT - 1),
                                )
                            o_ps[h] = ops

                        def emit_norm(h):
                            # h odd: normalize heads h-1, h
                            opsA, opsB = o_ps.pop(h - 1), o_ps.pop(h)
                            if dbg:
                                nc.vector.tensor_copy(den_dbg[0:1, h - 1, :], opsA[DV:DV + 1, :])
                                nc.vector.tensor_copy(den_dbg[0:1, h, :], opsB[DV:DV + 1, :])
                            nc.vector.tensor_copy(r2src[0:1, 0, :], opsA[DV:DV + 1, :])
                            nc.vector.tensor_copy(r2src[0:1, 1, :], opsB[DV:DV + 1, :])
                            nc.vector.reciprocal_approx_fast(
                                out=r2[0:1, 0, :], in_=r2src[0:1, 0, :])
                            nc.vector.reciprocal_approx_fast(
                                out=r2[0:1, 1, :], in_=r2src[0:1, 1, :])
                            if dbg:
                                nc.vector.tensor_copy(r_dbg[0:1, h // 2, :], r2[0:1, 0, :])
                                nc.vector.tensor_copy(r_dbg[64:65, h // 2, :], r2[0:1, 1, :])
                            nc.vector.tensor_copy(r2b[:, :, :], r2[:, :, :])
                            d = h // 2
                            rbcA = rpsum.tile([P, TPC], f32, tag="rb", name=f"rbA_{h}")
                            nc.tensor.matmul(
                                rbcA[:, :], bco[:, :], r2b[0:1, 0, :],
                                start=True, stop=True,
                            )
                            rbsA = rbp.tile([P, TPC], bf16, tag="rs", name=f"rsA_{h}")
                            nc.scalar.mul(rbsA[:, :], rbcA[:, :], 1.0)
                            nc.vector.tensor_tensor(
                                out=aO[0:DV, d, :], in0=opsA[0:DV, :],
                                in1=rbsA[0:DV, :], op=mybir.AluOpType.mult,
                            )
                            rbcB = rpsum.tile([P, TPC], f32, tag="rb", name=f"rbB_{h}")
                            nc.tensor.matmul(
                                rbcB[:, :], bco[:, :], r2b[0:1, 1, :],
                                start=True, stop=True,
                            )
                            rbsB = rbp.tile([P, TPC], bf16, tag="rs", name=f"rsB_{h}")
                            nc.scalar.mul(rbsB[:, :], rbcB[:, :], 1.0)
                            nc.vector.tensor_tensor(
                                out=aO[DV:P, d, :], in0=opsB[0:DV, :],
                                in1=rbsB[DV:P, :], op=mybir.AluOpType.mult,
                            )

                        def emit_hp(hp):
                            # S/exp for pair hp, with O matmuls of pair hp-1
                            # interleaved ahead of each S pair so PE never
                            # stalls on the exp ping-pong
                            ests = [est_bufs[(2 * hp) % 4], est_bufs[(2 * hp + 1) % 4]]
                            if hp > 0:
                                opsP = [
                                    opsum.tile([P, TPC], f32, tag="o",
                                               name=f"o_{2 * hp - 2}"),
                                    opsum.tile([P, TPC], f32, tag="o",
                                               name=f"o_{2 * hp - 1}"),
                                ]
                                estP = [est_bufs[(2 * hp - 2) % 4],
                                        est_bufs[(2 * hp - 1) % 4]]
                            for tp2 in range(NKP):
                                if hp > 0:
                                    for j in range(2):
                                        hP = 2 * hp - 2 + j
                                        for half in range(2):
                                            kt = 2 * tp2 + half
                                            nc.tensor.matmul(
                                                opsP[j][0:DV + 2, :],
                                                v2[:, hP, kt, 0:DV + 2],
                                                estP[j][:, kt, :],
                                                start=(kt == 0), stop=(kt == NKT - 1),
                                            )
                                sps = [
                                    spsum.tile([P, 2, TPC], f32, tag="sA",
                                               name=f"sA_{hp}_{tp2}"),
                                    spsum.tile([P, 2, TPC], f32, tag="sB",
                                               name=f"sB_{hp}_{tp2}"),
                                ]
                                for half in range(2):
                                    kt = 2 * tp2 + half
                                    for par in range(2):
                                        nc.tensor.matmul(
                                            sps[par][:, half, :],
                                            kT[64 * par:64 * par + 64, hp,
                                               kt * P:(kt + 1) * P],
                                            qT[64 * par:64 * par + 64, hp, :],
                                            start=True, stop=True,
                                            tile_position=(64 * par, 0),
                                        )
                                for par in range(2):
                                    if QA > 0:
                                        nc.scalar.activation(
                                            out=ests[par][:, 2 * tp2:2 * tp2 + 2, 0:QA],
                                            in_=sps[par][:, :, 0:QA],
                                            func=EXP, scale=0.125,
                                        )
                                    if QA < TPC:
                                        nc.vector.tensor_scalar(
                                            out=ests[par][:, 2 * tp2:2 * tp2 + 2, QA:TPC].bitcast(i8),
                                            in0=sps[par][:, :, QA:TPC],
                                            scalar1=SCH_MUL, scalar2=SCH_ADD,
                                            op0=mybir.AluOpType.mult,
                                            op1=mybir.AluOpType.add,
                                        )
                            if hp > 0:
                                o_ps[2 * hp - 2] = opsP[0]
                                o_ps[2 * hp - 1] = opsP[1]
                                emit_norm(2 * hp - 1)

                        for hp in range(NHP):
                            emit_hp(hp)
                        emit_o(H - 2)
                        emit_o(H - 1)
                        emit_norm(H - 1)

                        if dbg:
                            nc.sync.dma_start(out=qnT_d[:, :, :], in_=qnT)
                            nc.sync.dma_start(out=qT_d[:, :, :], in_=qT)
                            nc.sync.dma_start(out=kT_d[:, :, :], in_=kT)
                            nc.sync.dma_start(out=v2_d[:, :, :, :], in_=v2)
                            for j in range(4):
                                nc.sync.dma_start(out=est_d[:, j, :, :], in_=est_bufs[j])
                            nc.sync.dma_start(out=aO_d[:, :, :], in_=aO)
                            nc.sync.dma_start(out=den_d[:, :, :], in_=den_dbg)
                            nc.sync.dma_start(out=r_d[:, :, :], in_=r_dbg)

                    # =========== Phase 3: out projection + residual ============
                    with tc.tile_pool(name="p4o", bufs=2) as p4o, \
                         tc.tile_pool(name="fpsum", bufs=2, space="PSUM") as fpsum:
                        for tt in range(NTT):
                            for mc in range(2):
                                fps = fpsum.tile([P, TPC], f32, tag="f")
                                for dt in range(NDT):
                                    nc.tensor.matmul(
                                        fps,
                                        aO[:, dt, tt * P:(tt + 1) * P],
                                        wo_sb[:, dt, mc * 512:(mc + 1) * 512],
                                        start=(dt == 0), stop=(dt == NDT - 1),
                                    )
                                ob = p4o.tile([P, TPC], f32, tag="ob")
                                nc.vector.scalar_tensor_tensor(
                                    out=ob, in0=fps, scalar=1.0 / (WS * 32.0),
                                    in1=q_sb[:, tt, mc * 512:(mc + 1) * 512],
                                    op0=mybir.AluOpType.mult,
                                    op1=mybir.AluOpType.add,
                                )
                                nc.sync.dma_start(
                                    out=out_c[tt * P:(tt + 1) * P, mc * 512:(mc + 1) * 512],
                                    in_=ob,
                                )

    nc.compile()
    return nc


def _get_nc():
    if "nc" not in _CACHE:
        _CACHE["nc"] = build_nc()
    return _CACHE["nc"]


def make_in_maps(q, k, v, w_q, w_k, w_v, w_o, ln_g, ln_b):
    import ml_dtypes

    e4 = ml_dtypes.float8_e4m3
    q2 = np.ascontiguousarray(q.reshape(NT, D), dtype=np.float32)
    kT8 = np.ascontiguousarray(k.reshape(NT, D).T.astype(e4))
    vT8 = np.ascontiguousarray(v.reshape(NT, D).T.astype(e4))
    wgq = w_q * ln_g[None, :]
    wq8 = np.ascontiguousarray((WS * wgq).T.astype(e4))
    wk8 = np.ascontiguousarray((WS * w_k).T.astype(e4))
    wv8 = np.ascontiguousarray((WS * w_v).T.astype(e4))
    wo8 = np.ascontiguousarray((WS * w_o).T.astype(e4))
    cq = np.ascontiguousarray(w_q @ ln_b, dtype=np.float32)
    in_maps = []
    for c in range(N_CORES):
        sl = slice(c * TPC, (c + 1) * TPC)
        in_maps.append(
            {
                "q_c": q2[sl],
                "kT_c": np.ascontiguousarray(kT8[:, sl]),
                "vT_c": np.ascontiguousarray(vT8[:, sl]),
                "wq8": wq8,
                "wk8": wk8,
                "wv8": wv8,
                "wo8": wo8,
                "cq": cq,
            }
        )
    return in_maps


def run(inputs, trace=False, tmpdir=None):
    """Run the device kernel.  Returns (out [B, L, D], BassKernelResults)."""
    from concourse.bass_utils import run_bass_kernel_spmd

    nc = _get_nc()
    in_maps = make_in_maps(
        inputs["q"], inputs["k"], inputs["v"], inputs["w_q"], inputs["w_k"],
        inputs["w_v"], inputs["w_o"], inputs["ln_g"], inputs["ln_b"],
    )
    res = run_bass_kernel_spmd(
        nc, in_maps, list(range(N_CORES)), trace=trace, tmpdir=tmpdir
    )
    rows = np.concatenate([res.results[c]["out_c"] for c in range(N_CORES)], axis=0)
    return rows.reshape(B, L, D), res


def kernel(q, k, v, mask, w_q, w_k, w_v, w_o, ln_g, ln_b):
    q = np.asarray(q, dtype=np.float32)
    k = np.asarray(k, dtype=np.float32)
    v = np.asarray(v, dtype=np.float32)
    mask = np.asarray(mask)
    w_q = np.asarray(w_q, dtype=np.float32)
    w_k = np.asarray(w_k, dtype=np.float32)
    w_v = np.asarray(w_v, dtype=np.float32)
    w_o = np.asarray(w_o, dtype=np.float32)
    ln_g = np.asarray(ln_g, dtype=np.float32)
    ln_b = np.asarray(ln_b, dtype=np.float32)
    if not np.all(mask == 1):
        return _np_reference(q, k, v, mask, w_q, w_k, w_v, w_o, ln_g, ln_b)
    out, _ = run(
        {"q": q, "k": k, "v": v, "w_q": w_q, "w_k": w_k, "w_v": w_v,
         "w_o": w_o, "ln_g": ln_g, "ln_b": ln_b},
        trace=False,
    )
    return out


# revision 42
# speedup vs baseline: 1.1303x; 1.0618x over previous
"""MultiHeadAttention (pre-LN, residual) Trainium2 Bass kernel, 8 NeuronCores.

Problem: q,k,v [2, 2048, 1024], 16 heads x 64 dim, LN(q) -> QKV proj ->
softmax attention -> out proj -> +residual(q).

Sharding: core c owns tokens [512c, 512c+512) of the flattened [4096, 1024]
token axis (batch 0 = cores 0-3, batch 1 = cores 4-7).  Projections are
token-sharded; K^T / V are AllGathered (fp8, chunked) within each batch group
of 4 cores; each core runs attention + out projection for its 512 query
tokens over all 16 heads.

Matmuls are plain fp8 (1 output row / cycle; FWL weight loads).  S^T
contracts dk=64: the two heads of a pair run CONCURRENTLY in distinct PE
row-groups (tile_position (0,0) / (64,0)).  O^T contracts keys over 16
key-tiles with a 1/32-ones column (65th) in V producing the softmax
denominator in psum row 64.  exp() is split across engines by q columns:
ACT does true Exp into fp8e5m2 for q[0:QA]; DVE computes a Schraudolph-style
exp for q[QA:512] with one tensor_scalar (i8 = S*(4/ln2)/8 + 60) written
through an int8 bitcast of the e5m2 tile.  Normalization: approx-reciprocal
of psum row 64 + bf16 mask-matmul partition broadcast + DVE multiply.
"""

import numpy as np

N_CORES = 8
B, L, D = 2, 2048, 1024
H, DK, DV = 16, 64, 64
NT = B * L            # 4096 flattened tokens
TPC = NT // N_CORES   # 512 tokens per core
GROUP = 4             # cores per batch group
LB = L                # keys per batch (2048)
P = 128
NDT = D // P          # 8 d-tiles of 128
NTT = TPC // P        # 4 token tiles of 128 per core
NKT = LB // P         # 16 key tiles of 128 per batch
NKP = NKT // 2        # 8 key tile pairs
NHP = H // 2          # 8 head pairs
EPS = 1e-6

QA = 336              # q columns with true exp on ACT; rest Schraudolph on DVE
WS = 16.0             # host weight prescale into fp8 range
SCH_MUL = (4.0 / float(np.log(2.0))) / 8.0   # 0.72134752
SCH_ADD = 60.0        # e5m2 exponent bias 15 * 4

_CACHE = {}


def _np_reference(q, k, v, mask, w_q, w_k, w_v, w_o, ln_g, ln_b):
    """Pure-numpy fallback (only used if mask isn't all-ones)."""
    q64 = q.astype(np.float64)
    mu = q64.mean(-1, keepdims=True)
    var = q64.var(-1, keepdims=True)
    qn = (q64 - mu) / np.sqrt(var + EPS) * ln_g + ln_b
    Q = (qn @ w_q.T.astype(np.float64)).reshape(B, L, H, DK).transpose(0, 2, 1, 3)
    K = (k.astype(np.float64) @ w_k.T.astype(np.float64)).reshape(B, L, H, DK).transpose(0, 2, 1, 3)
    V = (v.astype(np.float64) @ w_v.T.astype(np.float64)).reshape(B, L, H, DV).transpose(0, 2, 1, 3)
    S = np.einsum("bhqd,bhkd->bhqk", Q / np.sqrt(DK), K)
    S = np.where(mask[None, None] == 0, -1e9, S)
    S = S - S.max(-1, keepdims=True)
    Pm = np.exp(S)
    Pm = Pm / Pm.sum(-1, keepdims=True)
    O = np.einsum("bhqk,bhkd->bhqd", Pm, V)
    O = O.transpose(0, 2, 1, 3).reshape(B, L, H * DV)
    out = O @ w_o.T.astype(np.float64) + q64
    return out.astype(np.float32)


def build_nc():
    import concourse.bass as bass
    import concourse.mybir as mybir
    import concourse.tile as tile
    from concourse import bacc
    from concourse.masks import make_identity

    f32 = mybir.dt.float32
    bf16 = mybir.dt.bfloat16
    f8e4 = mybir.dt.float8e4
    f8e5 = mybir.dt.float8e5
    i8 = mybir.dt.int8
    EXP = mybir.ActivationFunctionType.Exp

    nc = bacc.Bacc(num_devices=N_CORES)

    q_c = nc.declare_dram_parameter("q_c", [TPC, D], f32, isOutput=False)
    kT_c = nc.declare_dram_parameter("kT_c", [D, TPC], f8e4, isOutput=False)
    vT_c = nc.declare_dram_parameter("vT_c", [D, TPC], f8e4, isOutput=False)
    wq8 = nc.declare_dram_parameter("wq8", [D, D], f8e4, isOutput=False)
    wk8 = nc.declare_dram_parameter("wk8", [D, D], f8e4, isOutput=False)
    wv8 = nc.declare_dram_parameter("wv8", [D, D], f8e4, isOutput=False)
    wo8 = nc.declare_dram_parameter("wo8", [D, D], f8e4, isOutput=False)
    cq = nc.declare_dram_parameter("cq", [D], f32, isOutput=False)
    out_c = nc.declare_dram_parameter("out_c", [TPC, D], f32, isOutput=True)
    import os
    dbg = os.environ.get("KERNEL_DEBUG") == "1"
    if dbg:
        qnT_d = nc.declare_dram_parameter("qnT_d", [P, NDT, TPC], f8e4, isOutput=True)
        qT_d = nc.declare_dram_parameter("qT_d", [P, NHP, TPC], f8e4, isOutput=True)
        kT_d = nc.declare_dram_parameter("kT_d", [P, NHP, LB], f8e4, isOutput=True)
        v2_d = nc.declare_dram_parameter("v2_d", [P, H, NKT, 80], f8e5, isOutput=True)
        est_d = nc.declare_dram_parameter("est_d", [P, 4, NKT, TPC], f8e5, isOutput=True)
        aO_d = nc.declare_dram_parameter("aO_d", [P, NDT, TPC], f8e4, isOutput=True)
        den_d = nc.declare_dram_parameter("den_d", [1, H, TPC], f32, isOutput=True)
        r_d = nc.declare_dram_parameter("r_d", [65, NHP, TPC], f32, isOutput=True)

    RG = [[0, 1, 2, 3], [4, 5, 6, 7]]

    with tile.TileContext(nc) as tc:
        with tc.tile_pool(name="dram", bufs=1, space="DRAM") as dram:
            kag_in = dram.tile([D, TPC], f8e4, name="kag_in")
            kag_out = dram.tile([GROUP, D, TPC], f8e4, name="kag_out")
            vag_in = dram.tile([TPC, D], f8e5, name="vag_in")
            vag_out = dram.tile([GROUP, TPC, D], f8e5, name="vag_out")
            warm_in = dram.tile([64], f8e4, name="warm_in")
            warm_out = dram.tile([GROUP, 64], f8e4, name="warm_out")

            with tc.tile_pool(name="singles", bufs=1) as singles:
                ident = singles.tile([P, P], bf16)
                make_identity(nc, ident)
                # partition-broadcast mask: row0 -> out parts 0:64, row64 -> 64:128
                bcm = singles.tile([65, P], bf16)
                nc.vector.memset(bcm[0:64, :], 0.0)
                nc.vector.memset(bcm[64:65, 0:DK], 0.0)
                nc.vector.memset(bcm[0:1, 0:DK], 1.0)
                nc.vector.memset(bcm[64:65, DK:P], 1.0)
                bco = singles.tile([1, P], bf16)
                nc.vector.memset(bco, 1.0)
                r2 = singles.tile([1, 2, TPC], f32)
                r2src = singles.tile([1, 2, TPC], f32)
                r2b = singles.tile([1, 2, TPC], bf16)
                cq_sb = singles.tile([P, NDT], f32)
                nc.sync.dma_start(out=cq_sb, in_=cq.rearrange("(t p) -> p t", p=P))
                eps_sb = singles.tile([P, 1], f32)
                nc.vector.memset(eps_sb, EPS)

                with tc.tile_pool(name="persist", bufs=1) as persist:
                    q_sb = persist.tile([P, NTT, D], f32)          # residual + LN in
                    qn_sb = persist.tile([P, NTT, D], bf16)        # LN out
                    qnT = persist.tile([P, NDT, TPC], f8e4)        # qn^T
                    qT = persist.tile([P, NHP, TPC], f8e4)         # Q^T by head pair
                    kT = persist.tile([P, NHP, LB], f8e4)          # K^T full batch
                    v2 = persist.tile([P, H, NKT, 80], f8e5)       # V + 1/32 ones col
                    aO = persist.tile([P, NDT, TPC], f8e4)         # attn out (x32/den)
                    kag_sb = persist.tile([P, NDT, TPC], f8e4)     # K^T pre-AG staging
                    vag_sb = persist.tile([P, NTT, D], f8e5)       # V pre-AG staging
                    wk_sb = persist.tile([P, NDT, D], f8e4)
                    wv_sb = persist.tile([P, NDT, D], f8e4)
                    wq_sb = persist.tile([P, NDT, D], f8e4)
                    wo_sb = persist.tile([P, NDT, D], f8e4)
                    kc_sb = persist.tile([P, NDT, TPC], f8e4)      # k^T shard
                    vc_sb = persist.tile([P, NDT, TPC], f8e4)      # v^T shard

                    nc.gpsimd.collective_compute(
                        "AllGather", mybir.AluOpType.bypass,
                        replica_groups=RG,
                        ins=[warm_in[:].opt()],
                        outs=[warm_out[:, :].opt()],
                    )
                    # ones column of V (denominator): 1/32 so r = 32/den
                    nc.gpsimd.memset(v2[:, :, :, 64:65], 1.0 / 32.0)
                    nc.gpsimd.memset(v2[:, :, :, 65:66], 0.0)

                    # ---- input DMAs (K-proj operands first) ----
                    nc.sync.dma_start(out=wk_sb, in_=wk8.rearrange("(dt p) f -> p dt f", p=P))
                    nc.sync.dma_start(out=kc_sb, in_=kT_c.rearrange("(dt p) t -> p dt t", p=P))
                    nc.sync.dma_start(out=q_sb, in_=q_c.rearrange("(tt p) d -> p tt d", p=P))

                    # =========== Phase 1: LN + K proj/AG + Q proj + V proj/AG ===
                    with tc.tile_pool(name="p1s", bufs=4) as p1s, \
                         tc.tile_pool(name="ppsum", bufs=3, space="PSUM") as ppsum, \
                         tc.tile_pool(name="tpsum", bufs=2, space="PSUM") as tpsum:

                        # K projection: psum tile t = features [128t,128t+128)
                        for t in range(NDT):
                            ps = ppsum.tile([P, TPC], f32, tag="pp")
                            for dt in range(NDT):
                                nc.tensor.matmul(
                                    ps,
                                    wk_sb[:, dt, t * P:(t + 1) * P],
                                    kc_sb[:, dt, :],
                                    start=(dt == 0), stop=(dt == NDT - 1),
                                )
                            nc.scalar.mul(kag_sb[:, t, :], ps, 1.0 / WS)
                        nc.gpsimd.dma_start(
                            out=kag_in.rearrange("(hp p) t -> p hp t", p=P),
                            in_=kag_sb,
                        )
                        nc.gpsimd.collective_compute(
                            "AllGather", mybir.AluOpType.bypass,
                            replica_groups=RG,
                            ins=[kag_in[:, :].opt()],
                            outs=[kag_out[:, :, :].opt()],
                        )
                        for r in range(GROUP):
                            nc.sync.dma_start(
                                out=kT[:, :, r * TPC:(r + 1) * TPC],
                                in_=kag_out[r].rearrange("(hp p) t -> p hp t", p=P),
                            )

                        # LayerNorm on DVE (overlaps K proj on PE)
                        for tt in range(NTT):
                            stats = p1s.tile([P, 2, 6], f32, tag="st")
                            for sg in range(2):
                                nc.vector.bn_stats(
                                    out=stats[:, sg, :],
                                    in_=q_sb[:, tt, sg * 512:(sg + 1) * 512],
                                )
                            mv = p1s.tile([P, 2], f32, tag="mv")
                            nc.vector.bn_aggr(out=mv, in_=stats)
                            rstd = p1s.tile([P, 1], f32, tag="rs")
                            nc.scalar.activation(
                                out=rstd, in_=mv[:, 1:2],
                                func=mybir.ActivationFunctionType.Sqrt,
                                bias=eps_sb, scale=1.0,
                            )
                            nc.vector.reciprocal(out=rstd, in_=rstd)
                            nc.vector.tensor_scalar(
                                out=qn_sb[:, tt, :], in0=q_sb[:, tt, :],
                                scalar1=mv[:, 0:1], scalar2=rstd,
                                op0=mybir.AluOpType.subtract,
                                op1=mybir.AluOpType.mult,
                            )

                        nc.sync.dma_start(out=wv_sb, in_=wv8.rearrange("(dt p) f -> p dt f", p=P))
                        nc.sync.dma_start(out=vc_sb, in_=vT_c.rearrange("(dt p) t -> p dt t", p=P))
                        # V projection: psum tile (fc, tt) = [128 tok, 512 feat]
                        for fc in range(2):
                            for tt in range(NTT):
                                ps = ppsum.tile([P, TPC], f32, tag="pp")
                                for dt in range(NDT):
                                    nc.tensor.matmul(
                                        ps,
                                        vc_sb[:, dt, tt * P:(tt + 1) * P],
                                        wv_sb[:, dt, fc * 512:(fc + 1) * 512],
                                        start=(dt == 0), stop=(dt == NDT - 1),
                                    )
                                nc.vector.tensor_scalar_mul(
                                    vag_sb[:, tt, fc * 512:(fc + 1) * 512], ps, 1.0 / WS
                                )
                        nc.gpsimd.dma_start(
                            out=vag_in.rearrange("(tt p) f -> p tt f", p=P),
                            in_=vag_sb,
                        )
                        nc.gpsimd.collective_compute(
                            "AllGather", mybir.AluOpType.bypass,
                            replica_groups=RG,
                            ins=[vag_in[:, :].opt()],
                            outs=[vag_out[:, :, :].opt()],
                        )
                        # v2[p, h, kt, dv]; key = g*512 + tt*128 + p
                        nc.sync.dma_start(
                            out=v2[:, :, :, 0:DV],
                            in_=vag_out.rearrange(
                                "g (tt p) (h dv) -> p h (g tt) dv",
                                p=P, dv=DV,
                            ),
                        )

                        nc.sync.dma_start(out=wq_sb, in_=wq8.rearrange("(dt p) f -> p dt f", p=P))
                        nc.sync.dma_start(out=wo_sb, in_=wo8.rearrange("(dt p) m -> p dt m", p=P))
                        # qn^T: PE transpose (bf16) + ACT evict to fp8
                        for dt in range(NDT):
                            tp = tpsum.tile([P, TPC], bf16, tag="tp")
                            for tt in range(NTT):
                                nc.tensor.transpose(
                                    tp[:, tt * P:(tt + 1) * P],
                                    qn_sb[:, tt, dt * P:(dt + 1) * P],
                                    ident,
                                )
                            nc.scalar.mul(qnT[:, dt, :], tp, 1.0)

                        # Q projection + bias
                        for t in range(NDT):
                            ps = ppsum.tile([P, TPC], f32, tag="pp")
                            for dt in range(NDT):
                                nc.tensor.matmul(
                                    ps,
                                    wq_sb[:, dt, t * P:(t + 1) * P],
                                    qnT[:, dt, :],
                                    start=(dt == 0), stop=(dt == NDT - 1),
                                )
                            nc.scalar.activation(
                                out=qT[:, t, :], in_=ps,
                                func=mybir.ActivationFunctionType.Identity,
                                bias=cq_sb[:, t:t + 1], scale=1.0 / WS,
                            )

                    # =========== Phase 2: attention ============================
                    if dbg:
                        den_dbg = persist.tile([1, H, TPC], f32)
                        r_dbg = persist.tile([65, NHP, TPC], f32)
                    with tc.tile_pool(name="est", bufs=1) as estp, \
                         tc.tile_pool(name="rbp", bufs=2) as rbp, \
                         tc.tile_pool(name="spsum", bufs=1, space="PSUM") as spsum, \
                         tc.tile_pool(name="opsum", bufs=3, space="PSUM") as opsum, \
                         tc.tile_pool(name="rpsum", bufs=1, space="PSUM") as rpsum:

                        est_bufs = [estp.tile([P, NKT, TPC], f8e5, name=f"est{j}")
                                    for j in range(4)]
                        o_ps = {}

                        def emit_o(h):
                            est = est_bufs[h % 4]
                            ops = opsum.tile([P, TPC], f32, tag="o", name=f"o_{h}")
                            for kt in range(NKT):
                                nc.tensor.matmul(
                                    ops[0:DV + 2, :],
                                    v2[:, h, kt, 0:DV + 2],
                                    est[:, kt, :],
                                    start=(kt == 0), stop=(kt == NKT - 1),
                                )
                            o_ps[h] = ops

                        def emit_norm(h):
                            # h odd: normalize heads h-1, h
                            opsA, opsB = o_ps.pop(h - 1), o_ps.pop(h)
                            if dbg:
                                nc.vector.tensor_copy(den_dbg[0:1, h - 1, :], opsA[DV:DV + 1, :])
                                nc.vector.tensor_copy(den_dbg[0:1, h, :], opsB[DV:DV + 1, :])
                            nc.scalar.mul(r2src[0:1, 0, :], opsA[DV:DV + 1, :], 1.0)
                            nc.scalar.mul(r2src[0:1, 1, :], opsB[DV:DV + 1, :], 1.0)
                            nc.vector.reciprocal_approx_fast(
                                out=r2[0:1, :, :], in_=r2src[0:1, :, :])
                            if dbg:
                                nc.vector.tensor_copy(r_dbg[0:1, h // 2, :], r2[0:1, 0, :])
                                nc.vector.tensor_copy(r_dbg[64:65, h // 2, :], r2[0:1, 1, :])
                            nc.vector.tensor_copy(r2b[:, :, :], r2[:, :, :])
                            d = h // 2
                            rbcA = rpsum.tile([P, TPC], f32, tag="rb", name=f"rbA_{h}")
                            nc.tensor.matmul(
                                rbcA[:, :], bco[:, :], r2b[0:1, 0, :],
                                start=True, stop=True,
                            )
                            rbsA = rbp.tile([P, TPC], bf16, tag="rs", name=f"rsA_{h}")
                            nc.scalar.mul(rbsA[:, :], rbcA[:, :], 1.0)
                            nc.vector.tensor_tensor(
                                out=aO[0:DV, d, :], in0=opsA[0:DV, :],
                                in1=rbsA[0:DV, :], op=mybir.AluOpType.mult,
                            )
                            rbcB = rpsum.tile([P, TPC], f32, tag="rb", name=f"rbB_{h}")
                            nc.tensor.matmul(
                                rbcB[:, :], bco[:, :], r2b[0:1, 1, :],
                                start=True, stop=True,
                            )
                            rbsB = rbp.tile([P, TPC], bf16, tag="rs", name=f"rsB_{h}")
                            nc.scalar.mul(rbsB[:, :], rbcB[:, :], 1.0)
                            nc.vector.tensor_tensor(
                                out=aO[DV:P, d, :], in0=opsB[0:DV, :],
                                in1=rbsB[DV:P, :], op=mybir.AluOpType.mult,
                            )

                        def emit_hp(hp):
                            # S/exp for pair hp, with O matmuls of pair hp-1
                            # interleaved ahead of each S pair so PE never
                            # stalls on the exp ping-pong
                            ests = [est_bufs[(2 * hp) % 4], est_bufs[(2 * hp + 1) % 4]]
                            if hp > 0:
                                opsP = [
                                    opsum.tile([P, TPC], f32, tag="o",
                                               name=f"o_{2 * hp - 2}"),
                                    opsum.tile([P, TPC], f32, tag="o",
                                               name=f"o_{2 * hp - 1}"),
                                ]
                                estP = [est_bufs[(2 * hp - 2) % 4],
                                        est_bufs[(2 * hp - 1) % 4]]
                            for tp2 in range(NKP):
                                if hp > 0:
                                    for j in range(2):
                                        hP = 2 * hp - 2 + j
                                        for half in range(2):
                                            kt = 2 * tp2 + half
                                            nc.tensor.matmul(
                                                opsP[j][0:DV + 2, :],
                                                v2[:, hP, kt, 0:DV + 2],
                                                estP[j][:, kt, :],
                                                start=(kt == 0), stop=(kt == NKT - 1),
                                            )
                                sps = [
                                    spsum.tile([P, 2, TPC], f32, tag="sA",
                                               name=f"sA_{hp}_{tp2}"),
                                    spsum.tile([P, 2, TPC], f32, tag="sB",
                                               name=f"sB_{hp}_{tp2}"),
                                ]
                                for half in range(2):
                                    kt = 2 * tp2 + half
                                    for par in range(2):
                                        nc.tensor.matmul(
                                            sps[par][:, half, :],
                                            kT[64 * par:64 * par + 64, hp,
                                               kt * P:(kt + 1) * P],
                                            qT[64 * par:64 * par + 64, hp, :],
                                            start=True, stop=True,
                                            tile_position=(64 * par, 0),
                                        )
                                for par in range(2):
                                    if QA > 0:
                                        nc.scalar.activation(
                                            out=ests[par][:, 2 * tp2:2 * tp2 + 2, 0:QA],
                                            in_=sps[par][:, :, 0:QA],
                                            func=EXP, scale=0.125,
                                        )
                                    if QA < TPC:
                                        nc.vector.tensor_scalar(
                                            out=ests[par][:, 2 * tp2:2 * tp2 + 2, QA:TPC].bitcast(i8),
                                            in0=sps[par][:, :, QA:TPC],
                                            scalar1=SCH_MUL, scalar2=SCH_ADD,
                                            op0=mybir.AluOpType.mult,
                                            op1=mybir.AluOpType.add,
                                        )
                            if hp > 0:
                                o_ps[2 * hp - 2] = opsP[0]
                                o_ps[2 * hp - 1] = opsP[1]
                                emit_norm(2 * hp - 1)

                        for hp in range(NHP):
                            emit_hp(hp)
                        emit_o(H - 2)
                        emit_o(H - 1)
                        emit_norm(H - 1)

                        if dbg:
                            nc.sync.dma_start(out=qnT_d[:, :, :], in_=qnT)
                            nc.sync.dma_start(out=qT_d[:, :, :], in_=qT)
                            nc.sync.dma_start(out=kT_d[:, :, :], in_=kT)
                            nc.sync.dma_start(out=v2_d[:, :, :, :], in_=v2)
                            for j in range(4):
                                nc.sync.dma_start(out=est_d[:, j, :, :], in_=est_bufs[j])
                            nc.sync.dma_start(out=aO_d[:, :, :], in_=aO)
                            nc.sync.dma_start(out=den_d[:, :, :], in_=den_dbg)
                            nc.sync.dma_start(out=r_d[:, :, :], in_=r_dbg)

                    # =========== Phase 3: out projection + residual ============
                    with tc.tile_pool(name="p4o", bufs=2) as p4o, \
                         tc.tile_pool(name="fpsum", bufs=2, space="PSUM") as fpsum:
                        for tt in range(NTT):
                            for mc in range(2):
                                fps = fpsum.tile([P, TPC], f32, tag="f")
                                for dt in range(NDT):
                                    nc.tensor.matmul(
                                        fps,
                                        aO[:, dt, tt * P:(tt + 1) * P],
                                        wo_sb[:, dt, mc * 512:(mc + 1) * 512],
                                        start=(dt == 0), stop=(dt == NDT - 1),
                                    )
                                ob = p4o.tile([P, TPC], f32, tag="ob")
                                nc.vector.scalar_tensor_tensor(
                                    out=ob, in0=fps, scalar=1.0 / (WS * 32.0),
                                    in1=q_sb[:, tt, mc * 512:(mc + 1) * 512],
                                    op0=mybir.AluOpType.mult,
                                    op1=mybir.AluOpType.add,
                                )
                                nc.sync.dma_start(
                                    out=out_c[tt * P:(tt + 1) * P, mc * 512:(mc + 1) * 512],
                                    in_=ob,
                                )

    nc.compile()
    return nc


def _get_nc():
    if "nc" not in _CACHE:
        _CACHE["nc"] = build_nc()
    return _CACHE["nc"]


def make_in_maps(q, k, v, w_q, w_k, w_v, w_o, ln_g, ln_b):
    import ml_dtypes

    e4 = ml_dtypes.float8_e4m3
    q2 = np.ascontiguousarray(q.reshape(NT, D), dtype=np.float32)
    kT8 = np.ascontiguousarray(k.reshape(NT, D).T.astype(e4))
    vT8 = np.ascontiguousarray(v.reshape(NT, D).T.astype(e4))
    wgq = w_q * ln_g[None, :]
    wq8 = np.ascontiguousarray((WS * wgq).T.astype(e4))
    wk8 = np.ascontiguousarray((WS * w_k).T.astype(e4))
    wv8 = np.ascontiguousarray((WS * w_v).T.astype(e4))
    wo8 = np.ascontiguousarray((WS * w_o).T.astype(e4))
    cq = np.ascontiguousarray(w_q @ ln_b, dtype=np.float32)
    in_maps = []
    for c in range(N_CORES):
        sl = slice(c * TPC, (c + 1) * TPC)
        in_maps.append(
            {
                "q_c": q2[sl],
                "kT_c": np.ascontiguousarray(kT8[:, sl]),
                "vT_c": np.ascontiguousarray(vT8[:, sl]),
                "wq8": wq8,
                "wk8": wk8,
                "wv8": wv8,
                "wo8": wo8,
                "cq": cq,
            }
        )
    return in_maps


def run(inputs, trace=False, tmpdir=None):
    """Run the device kernel.  Returns (out [B, L, D], BassKernelResults)."""
    from concourse.bass_utils import run_bass_kernel_spmd

    nc = _get_nc()
    in_maps = make_in_maps(
        inputs["q"], inputs["k"], inputs["v"], inputs["w_q"], inputs["w_k"],
        inputs["w_v"], inputs["w_o"], inputs["ln_g"], inputs["ln_b"],
    )
    res = run_bass_kernel_spmd(
        nc, in_maps, list(range(N_CORES)), trace=trace, tmpdir=tmpdir
    )
    rows = np.concatenate([res.results[c]["out_c"] for c in range(N_CORES)], axis=0)
    return rows.reshape(B, L, D), res


def kernel(q, k, v, mask, w_q, w_k, w_v, w_o, ln_g, ln_b):
    q = np.asarray(q, dtype=np.float32)
    k = np.asarray(k, dtype=np.float32)
    v = np.asarray(v, dtype=np.float32)
    mask = np.asarray(mask)
    w_q = np.asarray(w_q, dtype=np.float32)
    w_k = np.asarray(w_k, dtype=np.float32)
    w_v = np.asarray(w_v, dtype=np.float32)
    w_o = np.asarray(w_o, dtype=np.float32)
    ln_g = np.asarray(ln_g, dtype=np.float32)
    ln_b = np.asarray(ln_b, dtype=np.float32)
    if not np.all(mask == 1):
        return _np_reference(q, k, v, mask, w_q, w_k, w_v, w_o, ln_g, ln_b)
    out, _ = run(
        {"q": q, "k": k, "v": v, "w_q": w_q, "w_k": w_k, "w_v": w_v,
         "w_o": w_o, "ln_g": ln_g, "ln_b": ln_b},
        trace=False,
    )
    return out


# revision 43
# speedup vs baseline: 1.1959x; 1.0581x over previous
"""MultiHeadAttention (pre-LN, residual) Trainium2 Bass kernel, 8 NeuronCores.

Problem: q,k,v [2, 2048, 1024], 16 heads x 64 dim, LN(q) -> QKV proj ->
softmax attention -> out proj -> +residual(q).

Sharding: core c owns tokens [512c, 512c+512) of the flattened [4096, 1024]
token axis (batch 0 = cores 0-3, batch 1 = cores 4-7).  Projections are
token-sharded; K^T / V are AllGathered (fp8, chunked) within each batch group
of 4 cores; each core runs attention + out projection for its 512 query
tokens over all 16 heads.

Matmuls are plain fp8 (1 output row / cycle; FWL weight loads).  S^T
contracts dk=64: the two heads of a pair run CONCURRENTLY in distinct PE
row-groups (tile_position (0,0) / (64,0)).  O^T contracts keys over 16
key-tiles with a 1/32-ones column (65th) in V producing the softmax
denominator in psum row 64.  exp() is split across engines by q columns:
ACT does true Exp into fp8e5m2 for q[0:QA]; DVE computes a Schraudolph-style
exp for q[QA:512] with one tensor_scalar (i8 = S*(4/ln2)/8 + 60) written
through an int8 bitcast of the e5m2 tile.  Normalization: approx-reciprocal
of psum row 64 + bf16 mask-matmul partition broadcast + DVE multiply.
"""

import numpy as np

N_CORES = 8
B, L, D = 2, 2048, 1024
H, DK, DV = 16, 64, 64
NT = B * L            # 4096 flattened tokens
TPC = NT // N_CORES   # 512 tokens per core
GROUP = 4             # cores per batch group
LB = L                # keys per batch (2048)
P = 128
NDT = D // P          # 8 d-tiles of 128
NTT = TPC // P        # 4 token tiles of 128 per core
NKT = LB // P         # 16 key tiles of 128 per batch
NKP = NKT // 2        # 8 key tile pairs
NHP = H // 2          # 8 head pairs
EPS = 1e-6

QA = 336              # q columns with true exp on ACT; rest Schraudolph on DVE
WS = 16.0             # host weight prescale into fp8 range
SCH_MUL = (4.0 / float(np.log(2.0))) / 8.0   # 0.72134752
SCH_ADD = 60.0        # e5m2 exponent bias 15 * 4

_CACHE = {}


def _np_reference(q, k, v, mask, w_q, w_k, w_v, w_o, ln_g, ln_b):
    """Pure-numpy fallback (only used if mask isn't all-ones)."""
    q64 = q.astype(np.float64)
    mu = q64.mean(-1, keepdims=True)
    var = q64.var(-1, keepdims=True)
    qn = (q64 - mu) / np.sqrt(var + EPS) * ln_g + ln_b
    Q = (qn @ w_q.T.astype(np.float64)).reshape(B, L, H, DK).transpose(0, 2, 1, 3)
    K = (k.astype(np.float64) @ w_k.T.astype(np.float64)).reshape(B, L, H, DK).transpose(0, 2, 1, 3)
    V = (v.astype(np.float64) @ w_v.T.astype(np.float64)).reshape(B, L, H, DV).transpose(0, 2, 1, 3)
    S = np.einsum("bhqd,bhkd->bhqk", Q / np.sqrt(DK), K)
    S = np.where(mask[None, None] == 0, -1e9, S)
    S = S - S.max(-1, keepdims=True)
    Pm = np.exp(S)
    Pm = Pm / Pm.sum(-1, keepdims=True)
    O = np.einsum("bhqk,bhkd->bhqd", Pm, V)
    O = O.transpose(0, 2, 1, 3).reshape(B, L, H * DV)
    out = O @ w_o.T.astype(np.float64) + q64
    return out.astype(np.float32)


def build_nc():
    import concourse.bass as bass
    import concourse.mybir as mybir
    import concourse.tile as tile
    from concourse import bacc
    from concourse.masks import make_identity

    f32 = mybir.dt.float32
    bf16 = mybir.dt.bfloat16
    f8e4 = mybir.dt.float8e4
    f8e5 = mybir.dt.float8e5
    i8 = mybir.dt.int8
    EXP = mybir.ActivationFunctionType.Exp

    nc = bacc.Bacc(num_devices=N_CORES)

    q_c = nc.declare_dram_parameter("q_c", [TPC, D], f32, isOutput=False)
    kT_c = nc.declare_dram_parameter("kT_c", [D, TPC], f8e4, isOutput=False)
    vT_c = nc.declare_dram_parameter("vT_c", [D, TPC], f8e4, isOutput=False)
    wq8 = nc.declare_dram_parameter("wq8", [D, D], f8e4, isOutput=False)
    wk8 = nc.declare_dram_parameter("wk8", [D, D], f8e4, isOutput=False)
    wv8 = nc.declare_dram_parameter("wv8", [D, D], f8e4, isOutput=False)
    wo8 = nc.declare_dram_parameter("wo8", [D, D], f8e4, isOutput=False)
    cq = nc.declare_dram_parameter("cq", [D], f32, isOutput=False)
    out_c = nc.declare_dram_parameter("out_c", [TPC, D], f32, isOutput=True)
    import os
    dbg = os.environ.get("KERNEL_DEBUG") == "1"
    if dbg:
        qnT_d = nc.declare_dram_parameter("qnT_d", [P, NDT, TPC], f8e4, isOutput=True)
        qT_d = nc.declare_dram_parameter("qT_d", [P, NHP, TPC], f8e4, isOutput=True)
        kT_d = nc.declare_dram_parameter("kT_d", [P, NHP, LB], f8e4, isOutput=True)
        v2_d = nc.declare_dram_parameter("v2_d", [P, H, NKT, 80], f8e5, isOutput=True)
        est_d = nc.declare_dram_parameter("est_d", [P, 4, NKT, TPC], f8e5, isOutput=True)
        aO_d = nc.declare_dram_parameter("aO_d", [P, NDT, TPC], f8e4, isOutput=True)
        den_d = nc.declare_dram_parameter("den_d", [1, H, TPC], f32, isOutput=True)
        r_d = nc.declare_dram_parameter("r_d", [65, NHP, TPC], f32, isOutput=True)

    RG = [[0, 1, 2, 3], [4, 5, 6, 7]]

    with tile.TileContext(nc) as tc:
        with tc.tile_pool(name="dram", bufs=1, space="DRAM") as dram:
            kag_in = dram.tile([D, TPC], f8e4, name="kag_in")
            kag_out = dram.tile([GROUP, D, TPC], f8e4, name="kag_out")
            vag_in = dram.tile([TPC, D], f8e5, name="vag_in")
            vag_out = dram.tile([GROUP, TPC, D], f8e5, name="vag_out")
            warm_in = dram.tile([64], f8e4, name="warm_in")
            warm_out = dram.tile([GROUP, 64], f8e4, name="warm_out")

            with tc.tile_pool(name="singles", bufs=1) as singles:
                ident = singles.tile([P, P], bf16)
                make_identity(nc, ident)
                # partition-broadcast mask: row0 -> out parts 0:64, row64 -> 64:128
                bcm = singles.tile([65, P], bf16)
                nc.vector.memset(bcm[0:64, :], 0.0)
                nc.vector.memset(bcm[64:65, 0:DK], 0.0)
                nc.vector.memset(bcm[0:1, 0:DK], 1.0)
                nc.vector.memset(bcm[64:65, DK:P], 1.0)
                bco = singles.tile([1, P], bf16)
                nc.vector.memset(bco, 1.0)
                r2 = singles.tile([1, 2, TPC], f32)
                r2src = singles.tile([1, 2, TPC], f32)
                r2b = singles.tile([1, 2, TPC], bf16)
                cq_sb = singles.tile([P, NDT], f32)
                nc.sync.dma_start(out=cq_sb, in_=cq.rearrange("(t p) -> p t", p=P))
                eps_sb = singles.tile([P, 1], f32)
                nc.vector.memset(eps_sb, EPS)

                with tc.tile_pool(name="persist", bufs=1) as persist:
                    q_sb = persist.tile([P, NTT, D], f32)          # residual + LN in
                    qn_sb = persist.tile([P, NTT, D], bf16)        # LN out
                    qnT = persist.tile([P, NDT, TPC], f8e4)        # qn^T
                    qT = persist.tile([P, 2, NHP, TPC], f8e4)      # Q^T zero-padded
                    kT = persist.tile([P, 2, NHP, LB], f8e4)       # K^T zero-padded
                    v2 = persist.tile([P, H, NKT, 80], f8e5)       # V + 1/32 ones col
                    aO = persist.tile([P, NDT, TPC], f8e4)         # attn out (x32/den)
                    kag_sb = persist.tile([P, NDT, TPC], f8e4)     # K^T pre-AG staging
                    vag_sb = persist.tile([P, NTT, D], f8e5)       # V pre-AG staging
                    wk_sb = persist.tile([P, NDT, D], f8e4)
                    wv_sb = persist.tile([P, NDT, D], f8e4)
                    wq_sb = persist.tile([P, NDT, D], f8e4)
                    wo_sb = persist.tile([P, NDT, D], f8e4)
                    kc_sb = persist.tile([P, NDT, TPC], f8e4)      # k^T shard
                    vc_sb = persist.tile([P, NDT, TPC], f8e4)      # v^T shard

                    nc.gpsimd.collective_compute(
                        "AllGather", mybir.AluOpType.bypass,
                        replica_groups=RG,
                        ins=[warm_in[:].opt()],
                        outs=[warm_out[:, :].opt()],
                    )
                    # zero pads of the K^T/Q^T head tiles (full-128 contraction)
                    nc.gpsimd.memset(kT[DK:P, 0, :, :], 0.0)
                    nc.gpsimd.memset(kT[0:DK, 1, :, :], 0.0)
                    nc.gpsimd.memset(qT[DK:P, 0, :, :], 0.0)
                    nc.gpsimd.memset(qT[0:DK, 1, :, :], 0.0)
                    # ones column of V (denominator): 1/32 so r = 32/den
                    nc.gpsimd.memset(v2[:, :, :, 64:65], 1.0 / 32.0)
                    nc.gpsimd.memset(v2[:, :, :, 65:66], 0.0)

                    # ---- input DMAs (K-proj operands first) ----
                    nc.sync.dma_start(out=wk_sb, in_=wk8.rearrange("(dt p) f -> p dt f", p=P))
                    nc.sync.dma_start(out=kc_sb, in_=kT_c.rearrange("(dt p) t -> p dt t", p=P))
                    nc.sync.dma_start(out=q_sb, in_=q_c.rearrange("(tt p) d -> p tt d", p=P))

                    # =========== Phase 1: LN + K proj/AG + Q proj + V proj/AG ===
                    with tc.tile_pool(name="p1s", bufs=4) as p1s, \
                         tc.tile_pool(name="ppsum", bufs=3, space="PSUM") as ppsum, \
                         tc.tile_pool(name="tpsum", bufs=2, space="PSUM") as tpsum:

                        # K projection: psum tile t = features [128t,128t+128)
                        for t in range(NDT):
                            ps = ppsum.tile([P, TPC], f32, tag="pp")
                            for dt in range(NDT):
                                nc.tensor.matmul(
                                    ps,
                                    wk_sb[:, dt, t * P:(t + 1) * P],
                                    kc_sb[:, dt, :],
                                    start=(dt == 0), stop=(dt == NDT - 1),
                                )
                            nc.scalar.mul(kag_sb[:, t, :], ps, 1.0 / WS)
                        nc.gpsimd.dma_start(
                            out=kag_in.rearrange("(hp p) t -> p hp t", p=P),
                            in_=kag_sb,
                        )
                        nc.gpsimd.collective_compute(
                            "AllGather", mybir.AluOpType.bypass,
                            replica_groups=RG,
                            ins=[kag_in[:, :].opt()],
                            outs=[kag_out[:, :, :].opt()],
                        )
                        for r in range(GROUP):
                            for par in range(2):
                                nc.sync.dma_start(
                                    out=kT[DK * par:DK * par + DK, par, :,
                                           r * TPC:(r + 1) * TPC],
                                    in_=kag_out[r].rearrange(
                                        "(hp par p) t -> p par hp t", par=2, p=DK,
                                    )[:, par, :, :],
                                )

                        # LayerNorm on DVE (overlaps K proj on PE)
                        for tt in range(NTT):
                            stats = p1s.tile([P, 2, 6], f32, tag="st")
                            for sg in range(2):
                                nc.vector.bn_stats(
                                    out=stats[:, sg, :],
                                    in_=q_sb[:, tt, sg * 512:(sg + 1) * 512],
                                )
                            mv = p1s.tile([P, 2], f32, tag="mv")
                            nc.vector.bn_aggr(out=mv, in_=stats)
                            rstd = p1s.tile([P, 1], f32, tag="rs")
                            nc.scalar.activation(
                                out=rstd, in_=mv[:, 1:2],
                                func=mybir.ActivationFunctionType.Sqrt,
                                bias=eps_sb, scale=1.0,
                            )
                            nc.vector.reciprocal(out=rstd, in_=rstd)
                            nc.vector.tensor_scalar(
                                out=qn_sb[:, tt, :], in0=q_sb[:, tt, :],
                                scalar1=mv[:, 0:1], scalar2=rstd,
                                op0=mybir.AluOpType.subtract,
                                op1=mybir.AluOpType.mult,
                            )

                        nc.sync.dma_start(out=wv_sb, in_=wv8.rearrange("(dt p) f -> p dt f", p=P))
                        nc.sync.dma_start(out=vc_sb, in_=vT_c.rearrange("(dt p) t -> p dt t", p=P))
                        # V projection: psum tile (fc, tt) = [128 tok, 512 feat]
                        for fc in range(2):
                            for tt in range(NTT):
                                ps = ppsum.tile([P, TPC], f32, tag="pp")
                                for dt in range(NDT):
                                    nc.tensor.matmul(
                                        ps,
                                        vc_sb[:, dt, tt * P:(tt + 1) * P],
                                        wv_sb[:, dt, fc * 512:(fc + 1) * 512],
                                        start=(dt == 0), stop=(dt == NDT - 1),
                                    )
                                nc.vector.tensor_scalar_mul(
                                    vag_sb[:, tt, fc * 512:(fc + 1) * 512], ps, 1.0 / WS
                                )
                        nc.gpsimd.dma_start(
                            out=vag_in.rearrange("(tt p) f -> p tt f", p=P),
                            in_=vag_sb,
                        )
                        nc.gpsimd.collective_compute(
                            "AllGather", mybir.AluOpType.bypass,
                            replica_groups=RG,
                            ins=[vag_in[:, :].opt()],
                            outs=[vag_out[:, :, :].opt()],
                        )
                        # v2[p, h, kt, dv]; key = g*512 + tt*128 + p
                        nc.sync.dma_start(
                            out=v2[:, :, :, 0:DV],
                            in_=vag_out.rearrange(
                                "g (tt p) (h dv) -> p h (g tt) dv",
                                p=P, dv=DV,
                            ),
                        )

                        nc.sync.dma_start(out=wq_sb, in_=wq8.rearrange("(dt p) f -> p dt f", p=P))
                        nc.sync.dma_start(out=wo_sb, in_=wo8.rearrange("(dt p) m -> p dt m", p=P))
                        # qn^T: PE transpose (bf16) + ACT evict to fp8
                        for dt in range(NDT):
                            tp = tpsum.tile([P, TPC], bf16, tag="tp")
                            for tt in range(NTT):
                                nc.tensor.transpose(
                                    tp[:, tt * P:(tt + 1) * P],
                                    qn_sb[:, tt, dt * P:(dt + 1) * P],
                                    ident,
                                )
                            nc.scalar.mul(qnT[:, dt, :], tp, 1.0)

                        # Q projection + bias
                        for t in range(NDT):
                            ps = ppsum.tile([P, TPC], f32, tag="pp")
                            for dt in range(NDT):
                                nc.tensor.matmul(
                                    ps,
                                    wq_sb[:, dt, t * P:(t + 1) * P],
                                    qnT[:, dt, :],
                                    start=(dt == 0), stop=(dt == NDT - 1),
                                )
                            nc.scalar.activation(
                                out=qT[0:DK, 0, t, :], in_=ps[0:DK, :],
                                func=mybir.ActivationFunctionType.Identity,
                                bias=cq_sb[0:DK, t:t + 1], scale=1.0 / WS,
                            )
                            nc.scalar.activation(
                                out=qT[DK:P, 1, t, :], in_=ps[DK:P, :],
                                func=mybir.ActivationFunctionType.Identity,
                                bias=cq_sb[DK:P, t:t + 1], scale=1.0 / WS,
                            )

                    # =========== Phase 2: attention ============================
                    if dbg:
                        den_dbg = persist.tile([1, H, TPC], f32)
                        r_dbg = persist.tile([65, NHP, TPC], f32)
                    with tc.tile_pool(name="est", bufs=1) as estp, \
                         tc.tile_pool(name="rbp", bufs=2) as rbp, \
                         tc.tile_pool(name="spsum", bufs=1, space="PSUM") as spsum, \
                         tc.tile_pool(name="opsum", bufs=3, space="PSUM") as opsum, \
                         tc.tile_pool(name="rpsum", bufs=1, space="PSUM") as rpsum:

                        est_bufs = [estp.tile([P, NKT, TPC], f8e5, name=f"est{j}")
                                    for j in range(4)]
                        o_ps = {}

                        def emit_o(h):
                            est = est_bufs[h % 4]
                            ops = opsum.tile([P, TPC], f32, tag="o", name=f"o_{h}")
                            for kt in range(NKT):
                                nc.tensor.matmul(
                                    ops[0:DV + 2, :],
                                    v2[:, h, kt, 0:DV + 2],
                                    est[:, kt, :],
                                    start=(kt == 0), stop=(kt == NKT - 1),
                                )
                            o_ps[h] = ops

                        def emit_norm(h):
                            # h odd: normalize heads h-1, h
                            opsA, opsB = o_ps.pop(h - 1), o_ps.pop(h)
                            if dbg:
                                nc.vector.tensor_copy(den_dbg[0:1, h - 1, :], opsA[DV:DV + 1, :])
                                nc.vector.tensor_copy(den_dbg[0:1, h, :], opsB[DV:DV + 1, :])
                            nc.scalar.mul(r2src[0:1, 0, :], opsA[DV:DV + 1, :], 1.0)
                            nc.scalar.mul(r2src[0:1, 1, :], opsB[DV:DV + 1, :], 1.0)
                            nc.vector.reciprocal_approx_fast(
                                out=r2[0:1, :, :], in_=r2src[0:1, :, :])
                            if dbg:
                                nc.vector.tensor_copy(r_dbg[0:1, h // 2, :], r2[0:1, 0, :])
                                nc.vector.tensor_copy(r_dbg[64:65, h // 2, :], r2[0:1, 1, :])
                            nc.vector.tensor_copy(r2b[:, :, :], r2[:, :, :])
                            d = h // 2
                            rbcA = rpsum.tile([P, TPC], f32, tag="rb", name=f"rbA_{h}")
                            nc.tensor.matmul(
                                rbcA[:, :], bco[:, :], r2b[0:1, 0, :],
                                start=True, stop=True,
                            )
                            rbsA = rbp.tile([P, TPC], bf16, tag="rs", name=f"rsA_{h}")
                            nc.scalar.mul(rbsA[:, :], rbcA[:, :], 1.0)
                            nc.vector.tensor_tensor(
                                out=aO[0:DV, d, :], in0=opsA[0:DV, :],
                                in1=rbsA[0:DV, :], op=mybir.AluOpType.mult,
                            )
                            rbcB = rpsum.tile([P, TPC], f32, tag="rb", name=f"rbB_{h}")
                            nc.tensor.matmul(
                                rbcB[:, :], bco[:, :], r2b[0:1, 1, :],
                                start=True, stop=True,
                            )
                            rbsB = rbp.tile([P, TPC], bf16, tag="rs", name=f"rsB_{h}")
                            nc.scalar.mul(rbsB[:, :], rbcB[:, :], 1.0)
                            nc.vector.tensor_tensor(
                                out=aO[DV:P, d, :], in0=opsB[0:DV, :],
                                in1=rbsB[DV:P, :], op=mybir.AluOpType.mult,
                            )

                        def emit_hp(hp):
                            # S/exp for pair hp, with O matmuls of pair hp-1
                            # interleaved ahead of each S pair so PE never
                            # stalls on the exp ping-pong
                            ests = [est_bufs[(2 * hp) % 4], est_bufs[(2 * hp + 1) % 4]]
                            if hp > 0:
                                opsP = [
                                    opsum.tile([P, TPC], f32, tag="o",
                                               name=f"o_{2 * hp - 2}"),
                                    opsum.tile([P, TPC], f32, tag="o",
                                               name=f"o_{2 * hp - 1}"),
                                ]
                                estP = [est_bufs[(2 * hp - 2) % 4],
                                        est_bufs[(2 * hp - 1) % 4]]
                            for tp2 in range(NKP):
                                if hp > 0:
                                    for j in range(2):
                                        hP = 2 * hp - 2 + j
                                        for half in range(2):
                                            kt = 2 * tp2 + half
                                            nc.tensor.matmul(
                                                opsP[j][0:DV + 2, :],
                                                v2[:, hP, kt, 0:DV + 2],
                                                estP[j][:, kt, :],
                                                start=(kt == 0), stop=(kt == NKT - 1),
                                            )
                                sps = [
                                    spsum.tile([P, 2, TPC], f32, tag="sA",
                                               name=f"sA_{hp}_{tp2}"),
                                    spsum.tile([P, 2, TPC], f32, tag="sB",
                                               name=f"sB_{hp}_{tp2}"),
                                ]
                                for half in range(2):
                                    kt = 2 * tp2 + half
                                    for par in range(2):
                                        nc.tensor.matmul(
                                            sps[par][:, half, :],
                                            kT[:, par, hp, kt * P:(kt + 1) * P],
                                            qT[:, par, hp, :],
                                            start=True, stop=True,
                                        )
                                for par in range(2):
                                    if QA > 0:
                                        nc.scalar.activation(
                                            out=ests[par][:, 2 * tp2:2 * tp2 + 2, 0:QA],
                                            in_=sps[par][:, :, 0:QA],
                                            func=EXP, scale=0.125,
                                        )
                                    if QA < TPC:
                                        nc.vector.tensor_scalar(
                                            out=ests[par][:, 2 * tp2:2 * tp2 + 2, QA:TPC].bitcast(i8),
                                            in0=sps[par][:, :, QA:TPC],
                                            scalar1=SCH_MUL, scalar2=SCH_ADD,
                                            op0=mybir.AluOpType.mult,
                                            op1=mybir.AluOpType.add,
                                        )
                            if hp > 0:
                                o_ps[2 * hp - 2] = opsP[0]
                                o_ps[2 * hp - 1] = opsP[1]
                                emit_norm(2 * hp - 1)

                        for hp in range(NHP):
                            emit_hp(hp)
                        emit_o(H - 2)
                        emit_o(H - 1)
                        emit_norm(H - 1)

                        if dbg:
                            nc.sync.dma_start(out=qnT_d[:, :, :], in_=qnT)
                            nc.sync.dma_start(out=qT_d[:, :, :], in_=qT)
                            nc.sync.dma_start(out=kT_d[:, :, :], in_=kT)
                            nc.sync.dma_start(out=v2_d[:, :, :, :], in_=v2)
                            for j in range(4):
                                nc.sync.dma_start(out=est_d[:, j, :, :], in_=est_bufs[j])
                            nc.sync.dma_start(out=aO_d[:, :, :], in_=aO)
                            nc.sync.dma_start(out=den_d[:, :, :], in_=den_dbg)
                            nc.sync.dma_start(out=r_d[:, :, :], in_=r_dbg)

                    # =========== Phase 3: out projection + residual ============
                    with tc.tile_pool(name="p4o", bufs=2) as p4o, \
                         tc.tile_pool(name="fpsum", bufs=2, space="PSUM") as fpsum:
                        for tt in range(NTT):
                            for mc in range(2):
                                fps = fpsum.tile([P, TPC], f32, tag="f")
                                for dt in range(NDT):
                                    nc.tensor.matmul(
                                        fps,
                                        aO[:, dt, tt * P:(tt + 1) * P],
                                        wo_sb[:, dt, mc * 512:(mc + 1) * 512],
                                        start=(dt == 0), stop=(dt == NDT - 1),
                                    )
                                ob = p4o.tile([P, TPC], f32, tag="ob")
                                nc.vector.scalar_tensor_tensor(
                                    out=ob, in0=fps, scalar=1.0 / (WS * 32.0),
                                    in1=q_sb[:, tt, mc * 512:(mc + 1) * 512],
                                    op0=mybir.AluOpType.mult,
                                    op1=mybir.AluOpType.add,
                                )
                                nc.sync.dma_start(
                                    out=out_c[tt * P:(tt + 1) * P, mc * 512:(mc + 1) * 512],
                                    in_=ob,
                                )

    nc.compile()
    return nc


def _get_nc():
    if "nc" not in _CACHE:
        _CACHE["nc"] = build_nc()
    return _CACHE["nc"]


def make_in_maps(q, k, v, w_q, w_k, w_v, w_o, ln_g, ln_b):
    import ml_dtypes

    e4 = ml_dtypes.float8_e4m3
    q2 = np.ascontiguousarray(q.reshape(NT, D), dtype=np.float32)
    kT8 = np.ascontiguousarray(k.reshape(NT, D).T.astype(e4))
    vT8 = np.ascontiguousarray(v.reshape(NT, D).T.astype(e4))
    wgq = w_q * ln_g[None, :]
    wq8 = np.ascontiguousarray((WS * wgq).T.astype(e4))
    wk8 = np.ascontiguousarray((WS * w_k).T.astype(e4))
    wv8 = np.ascontiguousarray((WS * w_v).T.astype(e4))
    wo8 = np.ascontiguousarray((WS * w_o).T.astype(e4))
    cq = np.ascontiguousarray(w_q @ ln_b, dtype=np.float32)
    in_maps = []
    for c in range(N_CORES):
        sl = slice(c * TPC, (c + 1) * TPC)
        in_maps.append(
            {
                "q_c": q2[sl],
                "kT_c": np.ascontiguousarray(kT8[:, sl]),
                "vT_c": np.ascontiguousarray(vT8[:, sl]),
                "wq8": wq8,
                "wk8": wk8,
                "wv8": wv8,
                "wo8": wo8,
                "cq": cq,
            }
        )
    return in_maps


def run(inputs, trace=False, tmpdir=None):
    """Run the device kernel.  Returns (out [B, L, D], BassKernelResults)."""
    from concourse.bass_utils import run_bass_kernel_spmd

    nc = _get_nc()
    in_maps = make_in_maps(
        inputs["q"], inputs["k"], inputs["v"], inputs["w_q"], inputs["w_k"],
        inputs["w_v"], inputs["w_o"], inputs["ln_g"], inputs["ln_b"],
    )
    res = run_bass_kernel_spmd(
        nc, in_maps, list(range(N_CORES)), trace=trace, tmpdir=tmpdir
    )
    rows = np.concatenate([res.results[c]["out_c"] for c in range(N_CORES)], axis=0)
    return rows.reshape(B, L, D), res


def kernel(q, k, v, mask, w_q, w_k, w_v, w_o, ln_g, ln_b):
    q = np.asarray(q, dtype=np.float32)
    k = np.asarray(k, dtype=np.float32)
    v = np.asarray(v, dtype=np.float32)
    mask = np.asarray(mask)
    w_q = np.asarray(w_q, dtype=np.float32)
    w_k = np.asarray(w_k, dtype=np.float32)
    w_v = np.asarray(w_v, dtype=np.float32)
    w_o = np.asarray(w_o, dtype=np.float32)
    ln_g = np.asarray(ln_g, dtype=np.float32)
    ln_b = np.asarray(ln_b, dtype=np.float32)
    if not np.all(mask == 1):
        return _np_reference(q, k, v, mask, w_q, w_k, w_v, w_o, ln_g, ln_b)
    out, _ = run(
        {"q": q, "k": k, "v": v, "w_q": w_q, "w_k": w_k, "w_v": w_v,
         "w_o": w_o, "ln_g": ln_g, "ln_b": ln_b},
        trace=False,
    )
    return out
